# revision 17
# baseline (speedup 1.0000x reference)
"""Trainium2 Bass kernel for nn_NestedParallelBlock (moe_routing).

Strategy: pure batch data-parallelism — batch 8 maps 1:1 onto the 8
NeuronCores, no collectives. Host-side prep is layout only: weights are
pre-transposed to contraction-major bf16 (norm1_g folded into the expand
weight), and the per-token nested-dims masks / router metadata are shipped as
tiny/boolean tensors. All heavy math (layernorms, both GEMMs, attention,
softmax, gelu, masking, residuals) runs on device.

Per-core pipeline (tuned for PE continuity / HAM warmth):
  LN1 (stats on DVE, apply on ACT) * host mask -> xbar-transpose to
  feature-major xnT -> GEMM1-k with PE column stats -> LN2-k -> GEMM1-v
  token-major with bn_stats LN2-v -> attention head-pairs woven with mlp
  GEMM1 groups at K-step granularity (QK k-major -> ACT exp -> PE row-sums ->
  col-tiled AV; reciprocals in [128,8] layout via DRAM bounce) -> batched
  in-place gelu -> GEMM2 with progressively streamed Wc^T -> masked halves
  accumulated onto an x-prefilled DRAM output via accum-DMA.
"""

from contextlib import ExitStack

import numpy as np
import ml_dtypes

import concourse.bass as bass
import concourse.mybir as mybir
import concourse.tile as tile
from concourse.bass_utils import run_bass_kernel_spmd
from concourse.vector_clock import ScopedClock, VectorClock
from concourse.masks import make_identity
from concourse.tile import add_dep_helper

F32 = mybir.dt.float32
BF16 = mybir.dt.bfloat16
AF = mybir.ActivationFunctionType
OP = mybir.AluOpType

B, N, D = 8, 1024, 1024
H, HD = 16, 64
NE = 4
EXPD = 7 * D          # 7168 expand output dim
CIN = 5 * D           # 5120 contract input dim
NT = N // 128         # 8 token tiles
ND = D // 128         # 8 feature tiles
NMLP = 4 * D // 128   # 32 mlp feature tiles
NC = CIN // 128       # 40 contract-input tiles
EPS = 1e-5

_wnop = [0]


def _split_multi_waits(nc):
    """This container's walrus accepts one sync-wait per instruction; hoist
    extra waits onto same-engine NoOps placed immediately before."""
    for f in nc.m.functions:
        for blk in f.blocks:
            out = []
            changed = False
            for inst in blk.instructions:
                si = getattr(inst, "sync_info", None)
                waits = list(si.on_wait) if si is not None else []
                if len(waits) > 1:
                    changed = True
                    for w in waits[:-1]:
                        _wnop[0] += 1
                        nop = mybir.InstNoOp(name=f"WNOP-{_wnop[0]}", ins=[], outs=[])
                        nop.engine = inst.engine
                        nop.sync_info = mybir.SyncInfo(on_wait=[w], on_update=[])
                        out.append(nop)
                    inst.sync_info = mybir.SyncInfo(
                        on_wait=[waits[-1]], on_update=list(si.on_update)
                    )
                out.append(inst)
            if changed:
                blk.instructions = out


class TC(tile.TileContext):
    def _drain_and_barrier(self, tick_clock, wait_clock):
        ticks = eval(str(tick_clock.global_clock).replace("VectorClock(", "").rstrip(")"))
        emitted = 0
        for p, t in enumerate(ticks):
            if t <= 0:
                continue
            c = VectorClock()
            c.require_at_least(p, t)
            d = self.nc.sync.drain()
            wait_clock.add_sem_waits(d.ins, ScopedClock({None: c}))
            if "wait" in str(d.ins):
                emitted += 1
        if emitted == 0:
            self.nc.sync.drain()
        self.nc.all_engine_barrier()
        popped = self.nc._tile_sem_poison_stack.pop()
        assert popped is self._sem_poison
        self.nc.clear_and_free_semaphores(list(self.sems.allocated().values()))
        self.nc.all_engine_barrier()

    def __exit__(self, *a):
        r = super().__exit__(*a)
        _split_multi_waits(self.nc)
        return r


def build_program(use_b1, use_g2b2, use_cb, use_alpha):
    nc = bass.Bass()
    dp = nc.declare_dram_parameter
    x_d = dp("x", [N, D], F32, isOutput=False)
    wT_d = dp("wT", [D, EXPD], BF16, isOutput=False)
    wcT_d = dp("wcT", [CIN, 2 * D], BF16, isOutput=False)
    im_d = dp("imask", [N, D], BF16, isOutput=False)
    om_d = dp("omask", [N, 2 * D], BF16, isOutput=False)
    mb_d = dp("mb", [4 * D], F32, isOutput=False)
    if use_alpha:
        psel_d = dp("psel", [N], F32, isOutput=False)
        alpha_d = dp("alpha", [1], F32, isOutput=False)
    if use_cb:
        cb_d = dp("cb", [2 * D], F32, isOutput=False)
    if use_g2b2:
        g2_d = dp("g2", [D], F32, isOutput=False)
        b2_d = dp("b2", [D], F32, isOutput=False)
    if use_b1:
        b4_d = dp("b4", [NE, EXPD], BF16, isOutput=False)
        ohT_d = dp("ohT", [NE, N], BF16, isOutput=False)
    out_d = dp("out", [N, D], F32, isOutput=True)
    # DRAM scratch for cross-layout bounces (row -> [128,8] -> broadcast)
    ksum_d = nc.dram_tensor("ksum_s", [N], F32)
    ksq_d = nc.dram_tensor("ksq_s", [N], F32)
    kmh_d = nc.dram_tensor("kmh_s", [N], F32)
    kih_d = nc.dram_tensor("kih_s", [N], F32)
    rs_d = nc.dram_tensor("rs_s", [H, N], F32)
    rr_d = nc.dram_tensor("rr_s", [H, N], F32)

    def COLI(v):
        return v.rearrange("(i p) -> p i", p=128)

    with TC(nc) as tc, ExitStack() as stk:
        # ------------------------------------------------ constants
        const = stk.enter_context(tc.tile_pool(name="const", bufs=1))
        mb_c = const.tile([128, NMLP], F32)
        nc.sync.dma_start(out=mb_c, in_=COLI(mb_d))
        eps_t = const.tile([128, 1], F32)
        nc.vector.memset(eps_t, EPS)
        zero_t = const.tile([128, 1], F32)
        nc.vector.memset(zero_t, 0.0)
        ones_bf = const.tile([128, 1], BF16)
        nc.vector.memset(ones_bf, 1.0)
        ident = const.tile([128, 128], BF16)
        make_identity(nc, ident)
        if use_alpha:
            psel_c = const.tile([128, NT], F32)
            nc.sync.dma_start(out=psel_c, in_=COLI(psel_d))
            alpha_c = const.tile([128, 1], F32)
            nc.sync.dma_start(out=alpha_c, in_=alpha_d[None, :].partition_broadcast(128))
            pscale_c = const.tile([128, NT], F32)
            nc.vector.tensor_scalar(pscale_c, psel_c, alpha_c, 1.0, OP.mult, OP.add)
        if use_cb:
            cb_c = const.tile([128, 2 * D], F32)
            nc.sync.dma_start(out=cb_c, in_=cb_d[None, :].partition_broadcast(128))
        if use_g2b2:
            g2_col = const.tile([128, ND], F32)
            nc.sync.dma_start(out=g2_col, in_=COLI(g2_d))
            b2_col = const.tile([128, ND], F32)
            nc.sync.dma_start(out=b2_col, in_=COLI(b2_d))
            g2_bc = const.tile([128, D], F32)
            nc.sync.dma_start(out=g2_bc, in_=g2_d[None, :].partition_broadcast(128))
            b2_bc = const.tile([128, D], F32)
            nc.sync.dma_start(out=b2_bc, in_=b2_d[None, :].partition_broadcast(128))
        if use_b1:
            b4_c = const.tile([NE, EXPD], BF16)
            nc.sync.dma_start(out=b4_c, in_=b4_d[:, :])
            ohT_c = const.tile([NE, N], BF16)
            nc.sync.dma_start(out=ohT_c, in_=ohT_d[:, :])

        # h^T, split so GEMM2 deps on attn vs mlp tiles stay independent
        hTp = stk.enter_context(tc.tile_pool(name="hTp", bufs=1))
        hTa = hTp.tile([128, ND, N], BF16)       # attn_out^T (c tiles 0..7)
        hTm1 = hTp.tile([128, 21, N], BF16)      # gelu(mlp)^T groups 0..20
        hTm2 = hTp.tile([128, NMLP - 21, N], BF16)  # gelu(mlp)^T groups 21..31

        def hTm_ap(m, isl=None):
            t = hTm1 if m < 21 else hTm2
            mm = m if m < 21 else m - 21
            if isl is None:
                return t[:, mm, :]
            return t[:, mm, isl]

        with tc.tile_pool(name="acts", bufs=1) as acts:
            xnT = acts.tile([128, ND, N], BF16)
            vtok = acts.tile([128, NT, D], BF16)
            kT = acts.tile([128, ND, N], BF16)

            # -------------------------------------------- LN1 + transpose
            with tc.tile_pool(name="ln1", bufs=3) as ln1, \
                 tc.tile_pool(name="tpp", bufs=4, space="PSUM") as tpp:
                for i in range(NT):
                    x_t = ln1.tile([128, D], F32, tag="xt")
                    nc.sync.dma_start(out=x_t, in_=x_d[i * 128:(i + 1) * 128, :])
                    im_t = ln1.tile([128, D], BF16, tag="im")
                    nc.sync.dma_start(out=im_t, in_=im_d[i * 128:(i + 1) * 128, :])
                    st = ln1.tile([128, 2, 6], F32, tag="st")
                    nc.vector.bn_stats(st[:, 0, :], x_t[:, 0:512])
                    nc.vector.bn_stats(st[:, 1, :], x_t[:, 512:1024])
                    mv = ln1.tile([128, 2], F32, tag="mv")
                    nc.vector.bn_aggr(mv, st)
                    istd = ln1.tile([128, 1], F32, tag="istd")
                    nc.scalar.activation(istd, mv[:, 1:2], AF.Sqrt, bias=eps_t)
                    nc.vector.reciprocal(istd, istd)
                    nbias = ln1.tile([128, 1], F32, tag="nbias")
                    nc.vector.tensor_scalar(nbias, mv[:, 0:1], istd, -1.0, OP.mult, OP.mult)
                    ln0 = ln1.tile([128, D], BF16, tag="ln0")
                    nc.scalar.activation(ln0, x_t, AF.Identity, bias=nbias, scale=istd)
                    xnm = ln1.tile([128, D], BF16, tag="xnm")
                    nc.vector.tensor_mul(xnm, ln0, im_t)
                    for g in range(2):
                        pt = tpp.tile([128, 512], BF16, tag="pt", name=f"pt{i}_{g}")
                        for jj in range(4):
                            j = 4 * g + jj
                            nc.tensor.transpose(
                                pt[:, jj * 128:(jj + 1) * 128],
                                xnm[:, j * 128:(j + 1) * 128], ident)
                        dst = xnT[:, 4 * g:4 * g + 4, i * 128:(i + 1) * 128]
                        src = pt.rearrange("p (j t) -> p j t", j=4)
                        if g == 0:
                            nc.vector.tensor_copy(dst, src)
                        else:
                            nc.scalar.copy(dst, src)

            with tc.tile_pool(name="g1w", bufs=3) as g1w, \
                 tc.tile_pool(name="wvp", bufs=1) as wvp:
                # prefetch the v weights early on the bulk queue
                wv = wvp.tile([128, ND, D], BF16)
                nc.sync.dma_start(
                    out=wv, in_=wT_d[:, 2 * D:3 * D].rearrange("(j p) o -> p j o", p=128))

                def load_w(o0):
                    w_t = g1w.tile([128, ND, 128], BF16, tag="w", name=f"w{o0}")
                    nc.sync.dma_start(
                        out=w_t,
                        in_=wT_d[:, o0:o0 + 128].rearrange("(j p) o -> p j o", p=128))
                    return w_t

                def fm_full(ps, w_t, o0):
                    for t2 in range(2):
                        for j in range(ND):
                            nc.tensor.matmul(
                                ps[:, t2 * 512:(t2 + 1) * 512], w_t[:, j, :],
                                xnT[:, j, t2 * 512:(t2 + 1) * 512],
                                start=(j == 0), stop=(j == ND - 1 and not use_b1))
                        if use_b1:
                            nc.tensor.matmul(
                                ps[:, t2 * 512:(t2 + 1) * 512], b4_c[:, o0:o0 + 128],
                                ohT_c[:, t2 * 512:(t2 + 1) * 512],
                                start=False, stop=True)

                def fm_steps(ps, w_t, o0):
                    """K-step emitters for a feature-major group into psum [128,N]"""
                    def step(j):
                        for t2 in range(2):
                            nc.tensor.matmul(
                                ps[:, t2 * 512:(t2 + 1) * 512], w_t[:, j, :],
                                xnT[:, j, t2 * 512:(t2 + 1) * 512],
                                start=(j == 0), stop=(j == ND - 1 and not use_b1))
                    def tail():
                        if use_b1:
                            for t2 in range(2):
                                nc.tensor.matmul(
                                    ps[:, t2 * 512:(t2 + 1) * 512], b4_c[:, o0:o0 + 128],
                                    ohT_c[:, t2 * 512:(t2 + 1) * 512],
                                    start=False, stop=True)
                    return step, tail

                # ---- k tiles + LN2-k via PE column stats
                with tc.tile_pool(name="kps", bufs=2, space="PSUM") as kps, \
                     tc.tile_pool(name="ksps", bufs=1, space="PSUM") as ksps, \
                     tc.tile_pool(name="kbuf", bufs=1) as kbuf, \
                     tc.tile_pool(name="krow", bufs=1) as krow:
                    kraw = kbuf.tile([128, ND, N], BF16)
                    k2 = kbuf.tile([128, ND, N], BF16)
                    for rk in range(ND):
                        w_t = load_w(D + rk * 128)
                        ps = kps.tile([128, N], F32, tag="ps", name=f"kps{rk}")
                        fm_full(ps, w_t, D + rk * 128)
                        nc.scalar.copy(kraw[:, rk, :], ps)
                        nc.vector.tensor_mul(k2[:, rk, :], kraw[:, rk, :], kraw[:, rk, :])
                    ks_s = [ksps.tile([1, 512], F32, tag=f"s{t2}", name=f"ks_s{t2}")
                            for t2 in range(2)]
                    ks_q = [ksps.tile([1, 512], F32, tag=f"q{t2}", name=f"ks_q{t2}")
                            for t2 in range(2)]
                    for rk in range(ND):
                        for t2 in range(2):
                            nc.tensor.matmul(
                                ks_s[t2], ones_bf, kraw[:, rk, t2 * 512:(t2 + 1) * 512],
                                start=(rk == 0), stop=(rk == ND - 1))
                            nc.tensor.matmul(
                                ks_q[t2], ones_bf, k2[:, rk, t2 * 512:(t2 + 1) * 512],
                                start=(rk == 0), stop=(rk == ND - 1))
                    # finish stats in [128, 8] layout via DRAM bounce
                    srow = krow.tile([1, N], F32)
                    qrow = krow.tile([1, N], F32)
                    for t2 in range(2):
                        sl = slice(t2 * 512, (t2 + 1) * 512)
                        nc.vector.tensor_copy(srow[:, sl], ks_s[t2])
                        nc.vector.tensor_copy(qrow[:, sl], ks_q[t2])
                    nc.gpsimd.dma_start(out=ksum_d[None, :], in_=srow)
                    nc.gpsimd.dma_start(out=ksq_d[None, :], in_=qrow)
                    sc = krow.tile([128, NT], F32)
                    nc.sync.dma_start(out=sc, in_=COLI(ksum_d))
                    qc = krow.tile([128, NT], F32)
                    nc.sync.dma_start(out=qc, in_=COLI(ksq_d))
                    mc = krow.tile([128, NT], F32)
                    nc.vector.tensor_scalar(mc, sc, 1.0 / D, None, OP.mult)
                    m2c = krow.tile([128, NT], F32)
                    nc.vector.tensor_mul(m2c, mc, mc)
                    vc = krow.tile([128, NT], F32)
                    nc.vector.tensor_scalar(vc, qc, 1.0 / D, None, OP.mult)
                    nc.vector.tensor_sub(vc, vc, m2c)
                    ic = krow.tile([128, NT], F32)
                    nc.scalar.activation(ic, vc, AF.Sqrt, bias=eps_t)
                    nc.vector.reciprocal(ic, ic)
                    nc.gpsimd.dma_start(out=COLI(kmh_d), in_=mc)
                    nc.gpsimd.dma_start(out=COLI(kih_d), in_=ic)
                    km_f = krow.tile([128, N], F32)
                    nc.sync.dma_start(out=km_f, in_=kmh_d[None, :].partition_broadcast(128))
                    ki_f = krow.tile([128, N], F32)
                    nc.sync.dma_start(out=ki_f, in_=kih_d[None, :].partition_broadcast(128))
                    kmean_bc = krow.tile([128, N], BF16)
                    nc.vector.tensor_copy(kmean_bc, km_f)
                    kistd_bc = krow.tile([128, N], BF16)
                    nc.vector.tensor_copy(kistd_bc, ki_f)
                    with tc.tile_pool(name="ksb", bufs=2) as ksb:
                        for rk in range(ND):
                            t1 = ksb.tile([128, N], BF16, tag="kap")
                            nc.vector.tensor_sub(t1, kraw[:, rk, :], kmean_bc)
                            if use_g2b2:
                                t3 = ksb.tile([128, N], BF16, tag="kap2")
                                nc.vector.tensor_mul(t3, t1, kistd_bc)
                                nc.scalar.activation(kT[:, rk, :], t3, AF.Identity,
                                                     bias=b2_col[:, rk:rk + 1],
                                                     scale=g2_col[:, rk:rk + 1])
                            else:
                                nc.vector.tensor_mul(kT[:, rk, :], t1, kistd_bc)

                # ---- v tiles, token-major + LN2-v
                with tc.tile_pool(name="vps", bufs=2, space="PSUM") as vps, \
                     tc.tile_pool(name="vsb", bufs=2) as vsb:
                    for i in range(NT):
                        ps = vps.tile([128, D], F32, tag="ps", name=f"vps{i}")
                        for j in range(ND):
                            for c2 in range(2):
                                nc.tensor.matmul(
                                    ps[:, c2 * 512:(c2 + 1) * 512],
                                    xnT[:, j, i * 128:(i + 1) * 128],
                                    wv[:, j, c2 * 512:(c2 + 1) * 512],
                                    start=(j == 0), stop=(j == ND - 1 and not use_b1))
                        if use_b1:
                            for c2 in range(2):
                                nc.tensor.matmul(
                                    ps[:, c2 * 512:(c2 + 1) * 512],
                                    ohT_c[:, i * 128:(i + 1) * 128],
                                    b4_c[:, 2 * D + c2 * 512:2 * D + (c2 + 1) * 512],
                                    start=False, stop=True)
                        vf = vsb.tile([128, D], F32, tag="vf")
                        nc.vector.tensor_copy(vf, ps)
                        st = vsb.tile([128, 2, 6], F32, tag="vst")
                        nc.vector.bn_stats(st[:, 0, :], vf[:, 0:512])
                        nc.vector.bn_stats(st[:, 1, :], vf[:, 512:1024])
                        mv = vsb.tile([128, 2], F32, tag="vmv")
                        nc.vector.bn_aggr(mv, st)
                        vistd = vsb.tile([128, 1], F32, tag="vistd")
                        nc.scalar.activation(vistd, mv[:, 1:2], AF.Sqrt, bias=eps_t)
                        nc.vector.reciprocal(vistd, vistd)
                        if use_g2b2:
                            vln = vsb.tile([128, D], BF16, tag="vln")
                            nc.vector.tensor_scalar(vln, vf, mv[:, 0:1], vistd,
                                                    OP.subtract, OP.mult)
                            v2 = vsb.tile([128, D], BF16, tag="vln2")
                            nc.vector.tensor_mul(v2, vln, g2_bc)
                            nc.vector.tensor_add(vtok[:, i, :], v2, b2_bc)
                        else:
                            nc.vector.tensor_scalar(vtok[:, i, :], vf, mv[:, 0:1],
                                                    vistd, OP.subtract, OP.mult)

                # ---- attention head-pairs woven with mlp groups
                with tc.tile_pool(name="qkp", bufs=2, space="PSUM") as qkp, \
                     tc.tile_pool(name="mlpp", bufs=2, space="PSUM") as mlpp, \
                     tc.tile_pool(name="qp", bufs=2) as qpp, \
                     tc.tile_pool(name="est", bufs=2) as estp, \
                     tc.tile_pool(name="rsb", bufs=2) as rsbp:

                    def mlp_group_full(m):
                        w_t = load_w(3 * D + m * 128)
                        ps = mlpp.tile([128, N], F32, tag="mlp", name=f"mfull{m}")
                        fm_full(ps, w_t, 3 * D + m * 128)
                        nc.scalar.activation(hTm_ap(m), ps, AF.Identity,
                                             bias=mb_c[:, m:m + 1], scale=1.0)

                    def rs_head(hh, est_h):
                        rps = qkp.tile([128, N], F32, tag="qk", name=f"rs{hh}")
                        for t2 in range(2):
                            for j in range(ND):
                                nc.tensor.matmul(
                                    rps[0:1, t2 * 512:(t2 + 1) * 512], ones_bf,
                                    est_h[:, j, t2 * 512:(t2 + 1) * 512],
                                    start=(j == 0), stop=(j == ND - 1))
                        rrow = rsbp.tile([1, N], F32, tag="rrow", name=f"rr{hh}")
                        nc.vector.tensor_copy(rrow, rps[0:1, :])
                        nc.gpsimd.dma_start(out=rs_d[hh][None, :], in_=rrow)
                        rcol = rsbp.tile([128, NT], F32, tag="rcol", name=f"rc{hh}")
                        nc.sync.dma_start(out=rcol, in_=COLI(rs_d[hh]))
                        nc.vector.reciprocal(rcol, rcol)
                        nc.gpsimd.dma_start(out=COLI(rr_d[hh]), in_=rcol)

                    def attn_pair(hp):
                        h0, h1 = 2 * hp, 2 * hp + 1
                        # lazy q^T rows for this pair
                        wq_t = load_w(hp * 128)
                        psq = mlpp.tile([128, N], F32, tag="mlp", name=f"qg{hp}")
                        fm_full(psq, wq_t, hp * 128)
                        qp = qpp.tile([128, N], BF16, tag="q", name=f"qp{hp}")
                        nc.vector.tensor_copy(qp, psq)
                        # two mlp groups woven into the QK/exp pipeline
                        mA, mB = 3 * hp, 3 * hp + 1
                        wA = load_w(3 * D + mA * 128)
                        wB = load_w(3 * D + mB * 128)
                        psA = mlpp.tile([128, N], F32, tag="mlp", name=f"mA{hp}")
                        psB = mlpp.tile([128, N], F32, tag="mlp", name=f"mB{hp}")
                        stepA, tailA = fm_steps(psA, wA, 3 * D + mA * 128)
                        stepB, tailB = fm_steps(psB, wB, 3 * D + mB * 128)
                        ests = {}
                        for hh in (h0, h1):
                            ests[hh] = estp.tile([128, ND, N], BF16, tag="est",
                                                 name=f"est{hh}")
                        last_exp = None
                        for j in range(ND):
                            for hh, hb in ((h0, 0), (h1, 64)):
                                ps = qkp.tile([128, N], F32, tag="qk", name=f"qk{hh}_{j}")
                                for qc in range(2):
                                    nc.tensor.matmul(
                                        ps[:, qc * 512:(qc + 1) * 512],
                                        kT[hb:hb + 64, hp, j * 128:(j + 1) * 128],
                                        qp[hb:hb + 64, qc * 512:(qc + 1) * 512],
                                        start=True, stop=True)
                                last_exp = nc.scalar.activation(
                                    ests[hh][:, j, :], ps, AF.Exp, scale=1.0 / 8.0)
                            stepA(j)
                            stepB(j)
                        # previous pair's gelus go here, after this pair's exps,
                        # so the ACT table switch never lands inside the weave
                        if hp >= 1:
                            for m in range(3 * (hp - 1), 3 * hp):
                                g = nc.scalar.activation(hTm_ap(m), hTm_ap(m),
                                                         AF.Gelu, bias=zero_t, scale=1.0)
                                add_dep_helper(g.ins, last_exp.ins,
                                               reason="gelu after pair exps")
                        tailA()
                        tailB()
                        nc.scalar.activation(hTm_ap(mA), psA, AF.Identity,
                                             bias=mb_c[:, mA:mA + 1], scale=1.0)
                        nc.scalar.activation(hTm_ap(mB), psB, AF.Identity,
                                             bias=mb_c[:, mB:mB + 1], scale=1.0)
                        # third mlp group as PE filler while exps drain
                        mlp_group_full(3 * hp + 2)
                        rs_head(h0, ests[h0])
                        rs_head(h1, ests[h1])
                        # AV, col-tiled per head pair
                        rsb = rsbp.tile([128, N], F32, tag="rsb", name=f"rsb{hp}")
                        nc.sync.dma_start(out=rsb[0:64, :],
                                            in_=rr_d[h0][None, :].partition_broadcast(64))
                        nc.sync.dma_start(out=rsb[64:128, :],
                                            in_=rr_d[h1][None, :].partition_broadcast(64))
                        psv = mlpp.tile([128, N], F32, tag="mlp", name=f"av{hp}")
                        for t2 in range(2):
                            sl = slice(t2 * 512, (t2 + 1) * 512)
                            for j in range(NT):
                                nc.tensor.matmul(
                                    psv[0:64, sl], vtok[:, j, hp * 128:hp * 128 + 64],
                                    ests[h0][:, j, sl],
                                    start=(j == 0), stop=(j == NT - 1))
                                nc.tensor.matmul(
                                    psv[64:128, sl],
                                    vtok[:, j, hp * 128 + 64:hp * 128 + 128],
                                    ests[h1][:, j, sl],
                                    start=(j == 0), stop=(j == NT - 1),
                                    tile_position=(0, 64))
                        nc.vector.tensor_mul(hTa[:, hp, :], psv, rsb)

                    def gelu_batch(ms):
                        for m in ms:
                            nc.scalar.activation(hTm_ap(m), hTm_ap(m),
                                                 AF.Gelu, bias=zero_t, scale=1.0)

                    for hp in range(NT):
                        attn_pair(hp)
                    # tail mlp groups; their gelus overlap the start of GEMM2
                    for m in range(24, 28):
                        mlp_group_full(m)
                    gelu_batch(range(21, 24))
                    for m in range(28, 32):
                        mlp_group_full(m)
                    gelu_batch(range(24, 32))

        # ------------------------------------------------ GEMM2 + combine
        # prefill out with x (residual base); masked z halves accumulate onto it
        nc.sync.dma_start(out=out_d[:, :], in_=x_d[:, :])
        # K-loop order: mlp tiles that gelu'd early first, attn tiles, last gelus
        jc_order = [8 + m for m in range(21)] + list(range(8)) + [8 + m for m in range(21, 32)]

        def lhs_g2(jc, i):
            isl = slice(i * 128, (i + 1) * 128)
            if jc < 8:
                return hTa[:, jc, isl]
            return hTm_ap(jc - 8, isl)

        with tc.tile_pool(name="g2w", bufs=2) as g2w, \
             tc.tile_pool(name="g2ps", bufs=6, space="PSUM") as g2ps, \
             tc.tile_pool(name="g2sb", bufs=6) as g2sb:
            for op2 in range(2):  # o2-chunk pairs: (z1a,z1b) then (z2a,z2b)
                wcs = []
                for oc in (2 * op2, 2 * op2 + 1):
                    w = g2w.tile([128, NC, 512], BF16, tag="wc", name=f"wc{oc}")
                    for jr in range(5):  # progressive 1MB sub-loads
                        nc.sync.dma_start(
                            out=w[:, jr * 8:(jr + 1) * 8, :],
                            in_=wcT_d[jr * 1024:(jr + 1) * 1024,
                                      oc * 512:(oc + 1) * 512].rearrange(
                                          "(j p) o -> p j o", p=128))
                    wcs.append(w)
                for i in range(NT):
                    pss = [g2ps.tile([128, 512], F32, tag="z", name=f"z{op2}_{i}_{u}")
                           for u in range(2)]
                    for nj, jc in enumerate(jc_order):
                        for u in range(2):
                            nc.tensor.matmul(pss[u], lhs_g2(jc, i), wcs[u][:, jc, :],
                                             start=(nj == 0), stop=(nj == NC - 1))
                    for u in range(2):
                        oc = 2 * op2 + u
                        om_t = g2sb.tile([128, 512], BF16, tag="om")
                        nc.sync.dma_start(
                            out=om_t,
                            in_=om_d[i * 128:(i + 1) * 128, oc * 512:(oc + 1) * 512])
                        zm = g2sb.tile([128, 512], F32, tag="zm")
                        if use_cb:
                            zb = g2sb.tile([128, 512], F32, tag="zb")
                            nc.vector.tensor_add(zb, pss[u], cb_c[:, oc * 512:(oc + 1) * 512])
                            nc.vector.tensor_mul(zm, zb, om_t)
                        else:
                            nc.vector.tensor_mul(zm, pss[u], om_t)
                        if op2 == 1 and use_alpha:
                            zs = g2sb.tile([128, 512], F32, tag="zs")
                            nc.vector.tensor_scalar(zs, zm, pscale_c[:, i:i + 1], None,
                                                    OP.mult)
                            zm = zs
                        dcol = (oc - 2) * 512 if op2 == 1 else oc * 512
                        nc.gpsimd.dma_start(
                            out=out_d[i * 128:(i + 1) * 128, dcol:dcol + 512],
                            in_=zm, accum_op=OP.add)
    return nc


_PROG_CACHE = {}


def prepare(x, expert_mask, router_probs, expand_weight, mlp_bias,
            contract_weight, contract_bias, norm1_g, norm1_b,
            norm2_g, norm2_b, alpha):
    """Build (program, per-core input maps) for the given full inputs."""
    x = np.asarray(x, np.float32)
    expert_mask = np.asarray(expert_mask, np.int32)
    router_probs = np.asarray(router_probs, np.float32)
    W = np.asarray(expand_weight, np.float32)
    mb = np.asarray(mlp_bias, np.float32)
    Wc = np.asarray(contract_weight, np.float32)
    cb = np.asarray(contract_bias, np.float32)
    g1 = np.asarray(norm1_g, np.float32)
    b1 = np.asarray(norm1_b, np.float32)
    g2 = np.asarray(norm2_g, np.float32)
    b2 = np.asarray(norm2_b, np.float32)
    alpha = np.asarray(alpha, np.float32)

    use_b1 = bool(np.any(b1 != 0))
    use_g2b2 = bool(np.any(b2 != 0) or np.any(g2 != 1))
    use_cb = bool(np.any(cb != 0))
    use_alpha = bool(np.any(alpha != 0))

    key = (use_b1, use_g2b2, use_cb, use_alpha)
    if key not in _PROG_CACHE:
        _PROG_CACHE[key] = build_program(*key)
    nc = _PROG_CACHE[key]

    wT = np.ascontiguousarray((W * g1[None, :]).T).astype(ml_dtypes.bfloat16)
    wcT = np.ascontiguousarray(Wc.T).astype(ml_dtypes.bfloat16)
    shift = (NE - 1 - expert_mask).astype(np.int64)
    d_in = (D >> shift)                    # [B, N]
    d_out = ((2 * D) >> shift)             # [B, N]
    imask = (np.arange(D)[None, None, :] < d_in[..., None]).astype(ml_dtypes.bfloat16)
    omask = (np.arange(2 * D)[None, None, :] < d_out[..., None]).astype(ml_dtypes.bfloat16)
    psel = np.take_along_axis(router_probs, expert_mask[..., None], axis=-1)[..., 0]

    in_maps = []
    for b in range(B):
        m = dict(x=np.ascontiguousarray(x[b]), wT=wT, wcT=wcT, mb=mb,
                 imask=np.ascontiguousarray(imask[b]),
                 omask=np.ascontiguousarray(omask[b]))
        if use_alpha:
            m["psel"] = np.ascontiguousarray(psel[b].astype(np.float32))
            m["alpha"] = alpha
        if use_cb:
            m["cb"] = cb
        if use_g2b2:
            m["g2"] = g2
            m["b2"] = b2
        if use_b1:
            mask_e = (np.arange(D)[None, :] < (D >> (NE - 1 - np.arange(NE)))[:, None])
            b4 = ((b1[None, :] * mask_e) @ W.T).astype(ml_dtypes.bfloat16)
            m["b4"] = b4
            ohT = np.zeros((NE, N), np.float32)
            ohT[expert_mask[b], np.arange(N)] = 1.0
            m["ohT"] = ohT.astype(ml_dtypes.bfloat16)
        in_maps.append(m)
    return nc, in_maps


def kernel(x, expert_mask, router_probs, expand_weight, mlp_bias,
           contract_weight, contract_bias, norm1_g, norm1_b,
           norm2_g, norm2_b, alpha):
    nc, in_maps = prepare(x, expert_mask, router_probs, expand_weight, mlp_bias,
                          contract_weight, contract_bias, norm1_g, norm1_b,
                          norm2_g, norm2_b, alpha)
    res = run_bass_kernel_spmd(nc, in_maps, list(range(B)))
    out = np.stack([res.results[b]["out"] for b in range(B)], axis=0)
    return (out, np.asarray(expert_mask, np.int32),
            np.asarray(router_probs, np.float32))


# revision 18
# speedup vs baseline: 1.1399x; 1.1399x over previous
"""Trainium2 Bass kernel for nn_NestedParallelBlock (moe_routing).

Strategy: pure batch data-parallelism — batch 8 maps 1:1 onto the 8
NeuronCores, no collectives. Host-side prep is layout only: weights are
pre-transposed to contraction-major bf16 (norm1_g folded into the expand
weight), and the per-token nested-dims masks / router metadata are shipped as
tiny/boolean tensors. All heavy math (layernorms, both GEMMs, attention,
softmax, gelu, masking, residuals) runs on device.

Per-core pipeline (tuned for PE continuity / HAM warmth):
  LN1 (stats on DVE, apply on ACT) * host mask -> xbar-transpose to
  feature-major xnT -> GEMM1-k with PE column stats -> LN2-k -> GEMM1-v
  token-major with bn_stats LN2-v -> attention head-pairs woven with mlp
  GEMM1 groups at K-step granularity (QK k-major -> ACT exp -> PE row-sums ->
  col-tiled AV; reciprocals in [128,8] layout via DRAM bounce) -> batched
  in-place gelu -> GEMM2 with progressively streamed Wc^T -> masked halves
  accumulated onto an x-prefilled DRAM output via accum-DMA.
"""

from contextlib import ExitStack

import numpy as np
import ml_dtypes

import concourse.bass as bass
import concourse.mybir as mybir
import concourse.tile as tile
from concourse.bass_utils import run_bass_kernel_spmd
from concourse.vector_clock import ScopedClock, VectorClock
from concourse.masks import make_identity
from concourse.tile import add_dep_helper

F32 = mybir.dt.float32
BF16 = mybir.dt.bfloat16
AF = mybir.ActivationFunctionType
OP = mybir.AluOpType

B, N, D = 8, 1024, 1024
H, HD = 16, 64
NE = 4
EXPD = 7 * D          # 7168 expand output dim
CIN = 5 * D           # 5120 contract input dim
NT = N // 128         # 8 token tiles
ND = D // 128         # 8 feature tiles
NMLP = 4 * D // 128   # 32 mlp feature tiles
NC = CIN // 128       # 40 contract-input tiles
EPS = 1e-5

_wnop = [0]


def _split_multi_waits(nc):
    """This container's walrus accepts one sync-wait per instruction; hoist
    extra waits onto same-engine NoOps placed immediately before."""
    for f in nc.m.functions:
        for blk in f.blocks:
            out = []
            changed = False
            for inst in blk.instructions:
                si = getattr(inst, "sync_info", None)
                waits = list(si.on_wait) if si is not None else []
                if len(waits) > 1:
                    changed = True
                    for w in waits[:-1]:
                        _wnop[0] += 1
                        nop = mybir.InstNoOp(name=f"WNOP-{_wnop[0]}", ins=[], outs=[])
                        nop.engine = inst.engine
                        nop.sync_info = mybir.SyncInfo(on_wait=[w], on_update=[])
                        out.append(nop)
                    inst.sync_info = mybir.SyncInfo(
                        on_wait=[waits[-1]], on_update=list(si.on_update)
                    )
                out.append(inst)
            if changed:
                blk.instructions = out


class TC(tile.TileContext):
    def _drain_and_barrier(self, tick_clock, wait_clock):
        ticks = eval(str(tick_clock.global_clock).replace("VectorClock(", "").rstrip(")"))
        emitted = 0
        for p, t in enumerate(ticks):
            if t <= 0:
                continue
            c = VectorClock()
            c.require_at_least(p, t)
            d = self.nc.sync.drain()
            wait_clock.add_sem_waits(d.ins, ScopedClock({None: c}))
            if "wait" in str(d.ins):
                emitted += 1
        if emitted == 0:
            self.nc.sync.drain()
        self.nc.all_engine_barrier()
        popped = self.nc._tile_sem_poison_stack.pop()
        assert popped is self._sem_poison
        self.nc.clear_and_free_semaphores(list(self.sems.allocated().values()))
        self.nc.all_engine_barrier()

    def __exit__(self, *a):
        r = super().__exit__(*a)
        _split_multi_waits(self.nc)
        return r


def build_program(use_b1, use_g2b2, use_cb, use_alpha):
    nc = bass.Bass()
    dp = nc.declare_dram_parameter
    x_d = dp("x", [N, D], F32, isOutput=False)
    wT_d = dp("wT", [D, EXPD], BF16, isOutput=False)
    wcT_d = dp("wcT", [CIN, 2 * D], BF16, isOutput=False)
    im_d = dp("imask", [N, D], BF16, isOutput=False)
    om_d = dp("omask", [N, 2 * D], BF16, isOutput=False)
    mb_d = dp("mb", [4 * D], F32, isOutput=False)
    if use_alpha:
        psel_d = dp("psel", [N], F32, isOutput=False)
        alpha_d = dp("alpha", [1], F32, isOutput=False)
    if use_cb:
        cb_d = dp("cb", [2 * D], F32, isOutput=False)
    if use_g2b2:
        g2_d = dp("g2", [D], F32, isOutput=False)
        b2_d = dp("b2", [D], F32, isOutput=False)
    if use_b1:
        b4_d = dp("b4", [NE, EXPD], BF16, isOutput=False)
        ohT_d = dp("ohT", [NE, N], BF16, isOutput=False)
    out_d = dp("out", [N, D], F32, isOutput=True)
    # DRAM scratch for cross-layout bounces (row -> [128,8] -> broadcast)
    ksum_d = nc.dram_tensor("ksum_s", [N], F32)
    ksq_d = nc.dram_tensor("ksq_s", [N], F32)
    kmh_d = nc.dram_tensor("kmh_s", [N], F32)
    kih_d = nc.dram_tensor("kih_s", [N], F32)
    rs_d = nc.dram_tensor("rs_s", [H, N], F32)
    rr_d = nc.dram_tensor("rr_s", [H, N], F32)

    def COLI(v):
        return v.rearrange("(i p) -> p i", p=128)

    with TC(nc) as tc, ExitStack() as stk:
        # ------------------------------------------------ constants
        const = stk.enter_context(tc.tile_pool(name="const", bufs=1))
        mb_c = const.tile([128, NMLP], F32)
        nc.sync.dma_start(out=mb_c, in_=COLI(mb_d))
        eps_t = const.tile([128, 1], F32)
        nc.vector.memset(eps_t, EPS)
        zero_t = const.tile([128, 1], F32)
        nc.vector.memset(zero_t, 0.0)
        ones_bf = const.tile([128, 1], BF16)
        nc.vector.memset(ones_bf, 1.0)
        ident = const.tile([128, 128], BF16)
        make_identity(nc, ident)
        if use_alpha:
            psel_c = const.tile([128, NT], F32)
            nc.sync.dma_start(out=psel_c, in_=COLI(psel_d))
            alpha_c = const.tile([128, 1], F32)
            nc.sync.dma_start(out=alpha_c, in_=alpha_d[None, :].partition_broadcast(128))
            pscale_c = const.tile([128, NT], F32)
            nc.vector.tensor_scalar(pscale_c, psel_c, alpha_c, 1.0, OP.mult, OP.add)
        if use_cb:
            cb_c = const.tile([128, 2 * D], F32)
            nc.sync.dma_start(out=cb_c, in_=cb_d[None, :].partition_broadcast(128))
        if use_g2b2:
            g2_col = const.tile([128, ND], F32)
            nc.sync.dma_start(out=g2_col, in_=COLI(g2_d))
            b2_col = const.tile([128, ND], F32)
            nc.sync.dma_start(out=b2_col, in_=COLI(b2_d))
            g2_bc = const.tile([128, D], F32)
            nc.sync.dma_start(out=g2_bc, in_=g2_d[None, :].partition_broadcast(128))
            b2_bc = const.tile([128, D], F32)
            nc.sync.dma_start(out=b2_bc, in_=b2_d[None, :].partition_broadcast(128))
        if use_b1:
            b4_c = const.tile([NE, EXPD], BF16)
            nc.sync.dma_start(out=b4_c, in_=b4_d[:, :])
            ohT_c = const.tile([NE, N], BF16)
            nc.sync.dma_start(out=ohT_c, in_=ohT_d[:, :])

        # h^T, split so GEMM2 deps on attn vs mlp tiles stay independent
        hTp = stk.enter_context(tc.tile_pool(name="hTp", bufs=1))
        hTa = hTp.tile([128, ND, N], BF16)       # attn_out^T (c tiles 0..7)
        hTm1 = hTp.tile([128, 21, N], BF16)      # gelu(mlp)^T groups 0..20
        hTm2 = hTp.tile([128, NMLP - 21, N], BF16)  # gelu(mlp)^T groups 21..31

        def hTm_ap(m, isl=None):
            t = hTm1 if m < 21 else hTm2
            mm = m if m < 21 else m - 21
            if isl is None:
                return t[:, mm, :]
            return t[:, mm, isl]

        with tc.tile_pool(name="acts", bufs=1) as acts:
            xnT = acts.tile([128, ND, N], BF16)
            vtok = acts.tile([128, NT, D], BF16)
            kT = acts.tile([128, ND, N], BF16)

            # -------------------------------------------- LN1 + transpose
            with tc.tile_pool(name="ln1", bufs=3) as ln1, \
                 tc.tile_pool(name="tpp", bufs=4, space="PSUM") as tpp:
                for i in range(NT):
                    x_t = ln1.tile([128, D], F32, tag="xt")
                    nc.sync.dma_start(out=x_t, in_=x_d[i * 128:(i + 1) * 128, :])
                    im_t = ln1.tile([128, D], BF16, tag="im")
                    nc.sync.dma_start(out=im_t, in_=im_d[i * 128:(i + 1) * 128, :])
                    st = ln1.tile([128, 2, 6], F32, tag="st")
                    nc.vector.bn_stats(st[:, 0, :], x_t[:, 0:512])
                    nc.vector.bn_stats(st[:, 1, :], x_t[:, 512:1024])
                    mv = ln1.tile([128, 2], F32, tag="mv")
                    nc.vector.bn_aggr(mv, st)
                    istd = ln1.tile([128, 1], F32, tag="istd")
                    nc.scalar.activation(istd, mv[:, 1:2], AF.Sqrt, bias=eps_t)
                    nc.vector.reciprocal(istd, istd)
                    nbias = ln1.tile([128, 1], F32, tag="nbias")
                    nc.vector.tensor_scalar(nbias, mv[:, 0:1], istd, -1.0, OP.mult, OP.mult)
                    ln0 = ln1.tile([128, D], BF16, tag="ln0")
                    nc.scalar.activation(ln0, x_t, AF.Identity, bias=nbias, scale=istd)
                    xnm = ln1.tile([128, D], BF16, tag="xnm")
                    nc.vector.tensor_mul(xnm, ln0, im_t)
                    for g in range(2):
                        pt = tpp.tile([128, 512], BF16, tag="pt", name=f"pt{i}_{g}")
                        for jj in range(4):
                            j = 4 * g + jj
                            nc.tensor.transpose(
                                pt[:, jj * 128:(jj + 1) * 128],
                                xnm[:, j * 128:(j + 1) * 128], ident)
                        dst = xnT[:, 4 * g:4 * g + 4, i * 128:(i + 1) * 128]
                        src = pt.rearrange("p (j t) -> p j t", j=4)
                        if g == 0:
                            nc.vector.tensor_copy(dst, src)
                        else:
                            nc.scalar.copy(dst, src)

            with tc.tile_pool(name="g1w", bufs=3) as g1w, \
                 tc.tile_pool(name="wvp", bufs=1) as wvp:
                # prefetch the v weights early on the bulk queue
                wv = wvp.tile([128, ND, D], BF16)
                nc.sync.dma_start(
                    out=wv, in_=wT_d[:, 2 * D:3 * D].rearrange("(j p) o -> p j o", p=128))

                def load_w(o0):
                    w_t = g1w.tile([128, ND, 128], BF16, tag="w", name=f"w{o0}")
                    nc.sync.dma_start(
                        out=w_t,
                        in_=wT_d[:, o0:o0 + 128].rearrange("(j p) o -> p j o", p=128))
                    return w_t

                def fm_full(ps, w_t, o0):
                    for t2 in range(2):
                        for j in range(ND):
                            nc.tensor.matmul(
                                ps[:, t2 * 512:(t2 + 1) * 512], w_t[:, j, :],
                                xnT[:, j, t2 * 512:(t2 + 1) * 512],
                                start=(j == 0), stop=(j == ND - 1 and not use_b1))
                        if use_b1:
                            nc.tensor.matmul(
                                ps[:, t2 * 512:(t2 + 1) * 512], b4_c[:, o0:o0 + 128],
                                ohT_c[:, t2 * 512:(t2 + 1) * 512],
                                start=False, stop=True)

                def fm_steps(ps, w_t, o0):
                    """K-step emitters for a feature-major group into psum [128,N]"""
                    def step(j):
                        for t2 in range(2):
                            nc.tensor.matmul(
                                ps[:, t2 * 512:(t2 + 1) * 512], w_t[:, j, :],
                                xnT[:, j, t2 * 512:(t2 + 1) * 512],
                                start=(j == 0), stop=(j == ND - 1 and not use_b1))
                    def tail():
                        if use_b1:
                            for t2 in range(2):
                                nc.tensor.matmul(
                                    ps[:, t2 * 512:(t2 + 1) * 512], b4_c[:, o0:o0 + 128],
                                    ohT_c[:, t2 * 512:(t2 + 1) * 512],
                                    start=False, stop=True)
                    return step, tail

                # ---- k tiles + LN2-k via PE column stats
                with tc.tile_pool(name="kps", bufs=2, space="PSUM") as kps, \
                     tc.tile_pool(name="ksps", bufs=1, space="PSUM") as ksps, \
                     tc.tile_pool(name="kbuf", bufs=1) as kbuf, \
                     tc.tile_pool(name="krow", bufs=1) as krow:
                    kraw = kbuf.tile([128, ND, N], BF16)
                    k2 = kbuf.tile([128, ND, N], BF16)
                    for rk in range(ND):
                        w_t = load_w(D + rk * 128)
                        ps = kps.tile([128, N], F32, tag="ps", name=f"kps{rk}")
                        fm_full(ps, w_t, D + rk * 128)
                        nc.scalar.copy(kraw[:, rk, :], ps)
                        nc.vector.tensor_mul(k2[:, rk, :], kraw[:, rk, :], kraw[:, rk, :])
                    ks_s = [ksps.tile([1, 512], F32, tag=f"s{t2}", name=f"ks_s{t2}")
                            for t2 in range(2)]
                    ks_q = [ksps.tile([1, 512], F32, tag=f"q{t2}", name=f"ks_q{t2}")
                            for t2 in range(2)]
                    for rk in range(ND):
                        for t2 in range(2):
                            nc.tensor.matmul(
                                ks_s[t2], ones_bf, kraw[:, rk, t2 * 512:(t2 + 1) * 512],
                                start=(rk == 0), stop=(rk == ND - 1))
                            nc.tensor.matmul(
                                ks_q[t2], ones_bf, k2[:, rk, t2 * 512:(t2 + 1) * 512],
                                start=(rk == 0), stop=(rk == ND - 1))
                    # finish stats in [128, 8] layout via DRAM bounce
                    srow = krow.tile([1, N], F32)
                    qrow = krow.tile([1, N], F32)
                    for t2 in range(2):
                        sl = slice(t2 * 512, (t2 + 1) * 512)
                        nc.vector.tensor_copy(srow[:, sl], ks_s[t2])
                        nc.vector.tensor_copy(qrow[:, sl], ks_q[t2])
                    nc.gpsimd.dma_start(out=ksum_d[None, :], in_=srow)
                    nc.gpsimd.dma_start(out=ksq_d[None, :], in_=qrow)
                    sc = krow.tile([128, NT], F32)
                    nc.sync.dma_start(out=sc, in_=COLI(ksum_d))
                    qc = krow.tile([128, NT], F32)
                    nc.sync.dma_start(out=qc, in_=COLI(ksq_d))
                    mc = krow.tile([128, NT], F32)
                    nc.vector.tensor_scalar(mc, sc, 1.0 / D, None, OP.mult)
                    m2c = krow.tile([128, NT], F32)
                    nc.vector.tensor_mul(m2c, mc, mc)
                    vc = krow.tile([128, NT], F32)
                    nc.vector.tensor_scalar(vc, qc, 1.0 / D, None, OP.mult)
                    nc.vector.tensor_sub(vc, vc, m2c)
                    ic = krow.tile([128, NT], F32)
                    nc.scalar.activation(ic, vc, AF.Sqrt, bias=eps_t)
                    nc.vector.reciprocal(ic, ic)
                    nc.gpsimd.dma_start(out=COLI(kmh_d), in_=mc)
                    nc.gpsimd.dma_start(out=COLI(kih_d), in_=ic)
                    km_f = krow.tile([128, N], F32)
                    nc.sync.dma_start(out=km_f, in_=kmh_d[None, :].partition_broadcast(128))
                    ki_f = krow.tile([128, N], F32)
                    nc.sync.dma_start(out=ki_f, in_=kih_d[None, :].partition_broadcast(128))
                    kmean_bc = krow.tile([128, N], BF16)
                    nc.vector.tensor_copy(kmean_bc, km_f)
                    kistd_bc = krow.tile([128, N], BF16)
                    nc.vector.tensor_copy(kistd_bc, ki_f)
                    with tc.tile_pool(name="ksb", bufs=2) as ksb:
                        for rk in range(ND):
                            t1 = ksb.tile([128, N], BF16, tag="kap")
                            nc.vector.tensor_sub(t1, kraw[:, rk, :], kmean_bc)
                            if use_g2b2:
                                t3 = ksb.tile([128, N], BF16, tag="kap2")
                                nc.vector.tensor_mul(t3, t1, kistd_bc)
                                nc.scalar.activation(kT[:, rk, :], t3, AF.Identity,
                                                     bias=b2_col[:, rk:rk + 1],
                                                     scale=g2_col[:, rk:rk + 1])
                            else:
                                nc.vector.tensor_mul(kT[:, rk, :], t1, kistd_bc)

                # ---- v tiles, token-major + LN2-v
                with tc.tile_pool(name="vps", bufs=2, space="PSUM") as vps, \
                     tc.tile_pool(name="vsb", bufs=2) as vsb:
                    for i in range(NT):
                        ps = vps.tile([128, D], F32, tag="ps", name=f"vps{i}")
                        for j in range(ND):
                            for c2 in range(2):
                                nc.tensor.matmul(
                                    ps[:, c2 * 512:(c2 + 1) * 512],
                                    xnT[:, j, i * 128:(i + 1) * 128],
                                    wv[:, j, c2 * 512:(c2 + 1) * 512],
                                    start=(j == 0), stop=(j == ND - 1 and not use_b1))
                        if use_b1:
                            for c2 in range(2):
                                nc.tensor.matmul(
                                    ps[:, c2 * 512:(c2 + 1) * 512],
                                    ohT_c[:, i * 128:(i + 1) * 128],
                                    b4_c[:, 2 * D + c2 * 512:2 * D + (c2 + 1) * 512],
                                    start=False, stop=True)
                        vf = vsb.tile([128, D], F32, tag="vf")
                        nc.vector.tensor_copy(vf, ps)
                        st = vsb.tile([128, 2, 6], F32, tag="vst")
                        nc.vector.bn_stats(st[:, 0, :], vf[:, 0:512])
                        nc.vector.bn_stats(st[:, 1, :], vf[:, 512:1024])
                        mv = vsb.tile([128, 2], F32, tag="vmv")
                        nc.vector.bn_aggr(mv, st)
                        vistd = vsb.tile([128, 1], F32, tag="vistd")
                        nc.scalar.activation(vistd, mv[:, 1:2], AF.Sqrt, bias=eps_t)
                        nc.vector.reciprocal(vistd, vistd)
                        if use_g2b2:
                            vln = vsb.tile([128, D], BF16, tag="vln")
                            nc.vector.tensor_scalar(vln, vf, mv[:, 0:1], vistd,
                                                    OP.subtract, OP.mult)
                            v2 = vsb.tile([128, D], BF16, tag="vln2")
                            nc.vector.tensor_mul(v2, vln, g2_bc)
                            nc.vector.tensor_add(vtok[:, i, :], v2, b2_bc)
                        else:
                            nc.vector.tensor_scalar(vtok[:, i, :], vf, mv[:, 0:1],
                                                    vistd, OP.subtract, OP.mult)

                # ---- attention head-pairs woven with mlp groups
                with tc.tile_pool(name="qkp", bufs=2, space="PSUM") as qkp, \
                     tc.tile_pool(name="mlpp", bufs=2, space="PSUM") as mlpp, \
                     tc.tile_pool(name="qp", bufs=2) as qpp, \
                     tc.tile_pool(name="est", bufs=2) as estp, \
                     tc.tile_pool(name="rsb", bufs=2) as rsbp:

                    def mlp_group_full(m):
                        w_t = load_w(3 * D + m * 128)
                        ps = mlpp.tile([128, N], F32, tag="mlp", name=f"mfull{m}")
                        fm_full(ps, w_t, 3 * D + m * 128)
                        nc.scalar.activation(hTm_ap(m), ps, AF.Identity,
                                             bias=mb_c[:, m:m + 1], scale=1.0)

                    def rs_head(hh, est_h):
                        rps = qkp.tile([128, N], F32, tag="qk", name=f"rs{hh}")
                        for t2 in range(2):
                            for j in range(ND):
                                nc.tensor.matmul(
                                    rps[0:1, t2 * 512:(t2 + 1) * 512], ones_bf,
                                    est_h[:, j, t2 * 512:(t2 + 1) * 512],
                                    start=(j == 0), stop=(j == ND - 1))
                        rrow = rsbp.tile([1, N], F32, tag="rrow", name=f"rr{hh}")
                        nc.vector.tensor_copy(rrow, rps[0:1, :])
                        nc.gpsimd.dma_start(out=rs_d[hh][None, :], in_=rrow)
                        rcol = rsbp.tile([128, NT], F32, tag="rcol", name=f"rc{hh}")
                        nc.sync.dma_start(out=rcol, in_=COLI(rs_d[hh]))
                        nc.vector.reciprocal(rcol, rcol)
                        nc.gpsimd.dma_start(out=COLI(rr_d[hh]), in_=rcol)

                    def attn_pair(hp):
                        h0, h1 = 2 * hp, 2 * hp + 1
                        # lazy q^T rows for this pair
                        wq_t = load_w(hp * 128)
                        psq = mlpp.tile([128, N], F32, tag="mlp", name=f"qg{hp}")
                        fm_full(psq, wq_t, hp * 128)
                        qp = qpp.tile([128, N], BF16, tag="q", name=f"qp{hp}")
                        nc.vector.tensor_copy(qp, psq)
                        # two mlp groups woven into the QK/exp pipeline
                        mA, mB = 3 * hp, 3 * hp + 1
                        wA = load_w(3 * D + mA * 128)
                        wB = load_w(3 * D + mB * 128)
                        psA = mlpp.tile([128, N], F32, tag="mlp", name=f"mA{hp}")
                        psB = mlpp.tile([128, N], F32, tag="mlp", name=f"mB{hp}")
                        stepA, tailA = fm_steps(psA, wA, 3 * D + mA * 128)
                        stepB, tailB = fm_steps(psB, wB, 3 * D + mB * 128)
                        ests = {}
                        for hh in (h0, h1):
                            ests[hh] = estp.tile([128, ND, N], BF16, tag="est",
                                                 name=f"est{hh}")
                        last_exp = None
                        for j in range(ND):
                            for hh, hb in ((h0, 0), (h1, 64)):
                                ps = qkp.tile([128, N], F32, tag="qk", name=f"qk{hh}_{j}")
                                for qc in range(2):
                                    nc.tensor.matmul(
                                        ps[:, qc * 512:(qc + 1) * 512],
                                        kT[hb:hb + 64, hp, j * 128:(j + 1) * 128],
                                        qp[hb:hb + 64, qc * 512:(qc + 1) * 512],
                                        start=True, stop=True)
                                last_exp = nc.scalar.activation(
                                    ests[hh][:, j, :], ps, AF.Exp, scale=1.0 / 8.0)
                            stepA(j)
                            stepB(j)
                        # previous pair's gelus go here, after this pair's exps,
                        # so the ACT table switch never lands inside the weave
                        if hp >= 1:
                            for m in range(3 * (hp - 1), 3 * hp):
                                g = nc.scalar.activation(hTm_ap(m), hTm_ap(m),
                                                         AF.Gelu, bias=zero_t, scale=1.0)
                                add_dep_helper(g.ins, last_exp.ins,
                                               reason="gelu after pair exps")
                        tailA()
                        tailB()
                        nc.scalar.activation(hTm_ap(mA), psA, AF.Identity,
                                             bias=mb_c[:, mA:mA + 1], scale=1.0)
                        nc.scalar.activation(hTm_ap(mB), psB, AF.Identity,
                                             bias=mb_c[:, mB:mB + 1], scale=1.0)
                        # third mlp group as PE filler while exps drain
                        mlp_group_full(3 * hp + 2)
                        rs_head(h0, ests[h0])
                        rs_head(h1, ests[h1])
                        # AV, col-tiled per head pair
                        rsb = rsbp.tile([128, N], F32, tag="rsb", name=f"rsb{hp}")
                        nc.sync.dma_start(out=rsb[0:64, :],
                                            in_=rr_d[h0][None, :].partition_broadcast(64))
                        nc.sync.dma_start(out=rsb[64:128, :],
                                            in_=rr_d[h1][None, :].partition_broadcast(64))
                        psv = qkp.tile([128, N], F32, tag="qk", name=f"av{hp}")
                        for t2 in range(2):
                            sl = slice(t2 * 512, (t2 + 1) * 512)
                            for j in range(NT):
                                nc.tensor.matmul(
                                    psv[0:64, sl], vtok[:, j, hp * 128:hp * 128 + 64],
                                    ests[h0][:, j, sl],
                                    start=(j == 0), stop=(j == NT - 1))
                                nc.tensor.matmul(
                                    psv[64:128, sl],
                                    vtok[:, j, hp * 128 + 64:hp * 128 + 128],
                                    ests[h1][:, j, sl],
                                    start=(j == 0), stop=(j == NT - 1),
                                    tile_position=(0, 64))
                        nc.vector.tensor_copy(hTa[:, hp, :], psv)

                    def gelu_batch(ms):
                        for m in ms:
                            nc.scalar.activation(hTm_ap(m), hTm_ap(m),
                                                 AF.Gelu, bias=zero_t, scale=1.0)

                    for hp in range(NT):
                        attn_pair(hp)
                    # tail mlp groups; their gelus overlap the start of GEMM2
                    for m in range(24, 28):
                        mlp_group_full(m)
                    gelu_batch(range(21, 24))
                    for m in range(28, 32):
                        mlp_group_full(m)
                    gelu_batch(range(24, 32))
                    # batched softmax normalization of the attention halves
                    for hp in range(NT):
                        rsb = rsbp.tile([128, N], F32, tag="rsb", name=f"rsbf{hp}")
                        nc.sync.dma_start(
                            out=rsb[0:64, :],
                            in_=rr_d[2 * hp][None, :].partition_broadcast(64))
                        nc.sync.dma_start(
                            out=rsb[64:128, :],
                            in_=rr_d[2 * hp + 1][None, :].partition_broadcast(64))
                        nc.vector.tensor_mul(hTa[:, hp, :], hTa[:, hp, :], rsb)

        # ------------------------------------------------ GEMM2 + combine
        # prefill out with x (residual base); masked z halves accumulate onto it
        nc.sync.dma_start(out=out_d[:, :], in_=x_d[:, :])
        # K-loop order: mlp tiles that gelu'd early first, attn tiles, last gelus
        jc_order = [8 + m for m in range(21)] + list(range(8)) + [8 + m for m in range(21, 32)]

        def lhs_g2(jc, i):
            isl = slice(i * 128, (i + 1) * 128)
            if jc < 8:
                return hTa[:, jc, isl]
            return hTm_ap(jc - 8, isl)

        with tc.tile_pool(name="g2w", bufs=2) as g2w, \
             tc.tile_pool(name="g2ps", bufs=6, space="PSUM") as g2ps, \
             tc.tile_pool(name="g2sb", bufs=6) as g2sb:
            for op2 in range(2):  # o2-chunk pairs: (z1a,z1b) then (z2a,z2b)
                wcs = []
                for oc in (2 * op2, 2 * op2 + 1):
                    w = g2w.tile([128, NC, 512], BF16, tag="wc", name=f"wc{oc}")
                    for jr in range(5):  # progressive 1MB sub-loads
                        nc.sync.dma_start(
                            out=w[:, jr * 8:(jr + 1) * 8, :],
                            in_=wcT_d[jr * 1024:(jr + 1) * 1024,
                                      oc * 512:(oc + 1) * 512].rearrange(
                                          "(j p) o -> p j o", p=128))
                    wcs.append(w)
                for i in range(NT):
                    pss = [g2ps.tile([128, 512], F32, tag="z", name=f"z{op2}_{i}_{u}")
                           for u in range(2)]
                    for nj, jc in enumerate(jc_order):
                        for u in range(2):
                            nc.tensor.matmul(pss[u], lhs_g2(jc, i), wcs[u][:, jc, :],
                                             start=(nj == 0), stop=(nj == NC - 1))
                    for u in range(2):
                        oc = 2 * op2 + u
                        om_t = g2sb.tile([128, 512], BF16, tag="om")
                        nc.sync.dma_start(
                            out=om_t,
                            in_=om_d[i * 128:(i + 1) * 128, oc * 512:(oc + 1) * 512])
                        zm = g2sb.tile([128, 512], F32, tag="zm")
                        if use_cb:
                            zb = g2sb.tile([128, 512], F32, tag="zb")
                            nc.vector.tensor_add(zb, pss[u], cb_c[:, oc * 512:(oc + 1) * 512])
                            nc.vector.tensor_mul(zm, zb, om_t)
                        else:
                            nc.vector.tensor_mul(zm, pss[u], om_t)
                        if op2 == 1 and use_alpha:
                            zs = g2sb.tile([128, 512], F32, tag="zs")
                            nc.vector.tensor_scalar(zs, zm, pscale_c[:, i:i + 1], None,
                                                    OP.mult)
                            zm = zs
                        dcol = (oc - 2) * 512 if op2 == 1 else oc * 512
                        nc.gpsimd.dma_start(
                            out=out_d[i * 128:(i + 1) * 128, dcol:dcol + 512],
                            in_=zm, accum_op=OP.add)
    return nc


_PROG_CACHE = {}


def prepare(x, expert_mask, router_probs, expand_weight, mlp_bias,
            contract_weight, contract_bias, norm1_g, norm1_b,
            norm2_g, norm2_b, alpha):
    """Build (program, per-core input maps) for the given full inputs."""
    x = np.asarray(x, np.float32)
    expert_mask = np.asarray(expert_mask, np.int32)
    router_probs = np.asarray(router_probs, np.float32)
    W = np.asarray(expand_weight, np.float32)
    mb = np.asarray(mlp_bias, np.float32)
    Wc = np.asarray(contract_weight, np.float32)
    cb = np.asarray(contract_bias, np.float32)
    g1 = np.asarray(norm1_g, np.float32)
    b1 = np.asarray(norm1_b, np.float32)
    g2 = np.asarray(norm2_g, np.float32)
    b2 = np.asarray(norm2_b, np.float32)
    alpha = np.asarray(alpha, np.float32)

    use_b1 = bool(np.any(b1 != 0))
    use_g2b2 = bool(np.any(b2 != 0) or np.any(g2 != 1))
    use_cb = bool(np.any(cb != 0))
    use_alpha = bool(np.any(alpha != 0))

    key = (use_b1, use_g2b2, use_cb, use_alpha)
    if key not in _PROG_CACHE:
        _PROG_CACHE[key] = build_program(*key)
    nc = _PROG_CACHE[key]

    wT = np.ascontiguousarray((W * g1[None, :]).T).astype(ml_dtypes.bfloat16)
    wcT = np.ascontiguousarray(Wc.T).astype(ml_dtypes.bfloat16)
    shift = (NE - 1 - expert_mask).astype(np.int64)
    d_in = (D >> shift)                    # [B, N]
    d_out = ((2 * D) >> shift)             # [B, N]
    imask = (np.arange(D)[None, None, :] < d_in[..., None]).astype(ml_dtypes.bfloat16)
    omask = (np.arange(2 * D)[None, None, :] < d_out[..., None]).astype(ml_dtypes.bfloat16)
    psel = np.take_along_axis(router_probs, expert_mask[..., None], axis=-1)[..., 0]

    in_maps = []
    for b in range(B):
        m = dict(x=np.ascontiguousarray(x[b]), wT=wT, wcT=wcT, mb=mb,
                 imask=np.ascontiguousarray(imask[b]),
                 omask=np.ascontiguousarray(omask[b]))
        if use_alpha:
            m["psel"] = np.ascontiguousarray(psel[b].astype(np.float32))
            m["alpha"] = alpha
        if use_cb:
            m["cb"] = cb
        if use_g2b2:
            m["g2"] = g2
            m["b2"] = b2
        if use_b1:
            mask_e = (np.arange(D)[None, :] < (D >> (NE - 1 - np.arange(NE)))[:, None])
            b4 = ((b1[None, :] * mask_e) @ W.T).astype(ml_dtypes.bfloat16)
            m["b4"] = b4
            ohT = np.zeros((NE, N), np.float32)
            ohT[expert_mask[b], np.arange(N)] = 1.0
            m["ohT"] = ohT.astype(ml_dtypes.bfloat16)
        in_maps.append(m)
    return nc, in_maps


def kernel(x, expert_mask, router_probs, expand_weight, mlp_bias,
           contract_weight, contract_bias, norm1_g, norm1_b,
           norm2_g, norm2_b, alpha):
    nc, in_maps = prepare(x, expert_mask, router_probs, expand_weight, mlp_bias,
                          contract_weight, contract_bias, norm1_g, norm1_b,
                          norm2_g, norm2_b, alpha)
    res = run_bass_kernel_spmd(nc, in_maps, list(range(B)))
    out = np.stack([res.results[b]["out"] for b in range(B)], axis=0)
    return (out, np.asarray(expert_mask, np.int32),
            np.asarray(router_probs, np.float32))


# revision 20
# speedup vs baseline: 1.4150x; 1.2414x over previous
"""Trainium2 Bass kernel for nn_NestedParallelBlock (moe_routing).

Strategy: pure batch data-parallelism — batch 8 maps 1:1 onto the 8
NeuronCores, no collectives. Host-side prep is layout only: weights are
pre-transposed to contraction-major bf16 (norm1_g folded into the expand
weight), and the per-token nested-dims masks / router metadata are shipped as
tiny/boolean tensors. All heavy math (layernorms, both GEMMs, attention,
softmax, gelu, masking, residuals) runs on device.

Per-core pipeline (tuned for PE continuity / HAM warmth):
  LN1 (stats on DVE, apply on ACT) * host mask -> xbar-transpose to
  feature-major xnT -> GEMM1-k with PE column stats -> LN2-k -> GEMM1-v
  token-major with bn_stats LN2-v -> attention head-pairs woven with mlp
  GEMM1 groups at K-step granularity (QK k-major -> ACT exp -> PE row-sums ->
  col-tiled AV; reciprocals in [128,8] layout via DRAM bounce) -> batched
  in-place gelu -> GEMM2 with progressively streamed Wc^T -> masked halves
  accumulated onto an x-prefilled DRAM output via accum-DMA.
"""

from contextlib import ExitStack

import numpy as np
import ml_dtypes

import concourse.bass as bass
import concourse.mybir as mybir
import concourse.tile as tile
from concourse.bass_utils import run_bass_kernel_spmd
from concourse.vector_clock import ScopedClock, VectorClock
from concourse.masks import make_identity
from concourse.tile import add_dep_helper

F32 = mybir.dt.float32
BF16 = mybir.dt.bfloat16
AF = mybir.ActivationFunctionType
OP = mybir.AluOpType

B, N, D = 8, 1024, 1024
H, HD = 16, 64
NE = 4
EXPD = 7 * D          # 7168 expand output dim
CIN = 5 * D           # 5120 contract input dim
NT = N // 128         # 8 token tiles
ND = D // 128         # 8 feature tiles
NMLP = 4 * D // 128   # 32 mlp feature tiles
NC = CIN // 128       # 40 contract-input tiles
EPS = 1e-5

_wnop = [0]


def _split_multi_waits(nc):
    """This container's walrus accepts one sync-wait per instruction; hoist
    extra waits onto same-engine NoOps placed immediately before."""
    for f in nc.m.functions:
        for blk in f.blocks:
            out = []
            changed = False
            for inst in blk.instructions:
                si = getattr(inst, "sync_info", None)
                waits = list(si.on_wait) if si is not None else []
                if len(waits) > 1:
                    changed = True
                    for w in waits[:-1]:
                        _wnop[0] += 1
                        nop = mybir.InstNoOp(name=f"WNOP-{_wnop[0]}", ins=[], outs=[])
                        nop.engine = inst.engine
                        nop.sync_info = mybir.SyncInfo(on_wait=[w], on_update=[])
                        out.append(nop)
                    inst.sync_info = mybir.SyncInfo(
                        on_wait=[waits[-1]], on_update=list(si.on_update)
                    )
                out.append(inst)
            if changed:
                blk.instructions = out


class TC(tile.TileContext):
    def _drain_and_barrier(self, tick_clock, wait_clock):
        ticks = eval(str(tick_clock.global_clock).replace("VectorClock(", "").rstrip(")"))
        emitted = 0
        for p, t in enumerate(ticks):
            if t <= 0:
                continue
            c = VectorClock()
            c.require_at_least(p, t)
            d = self.nc.sync.drain()
            wait_clock.add_sem_waits(d.ins, ScopedClock({None: c}))
            if "wait" in str(d.ins):
                emitted += 1
        if emitted == 0:
            self.nc.sync.drain()
        self.nc.all_engine_barrier()
        popped = self.nc._tile_sem_poison_stack.pop()
        assert popped is self._sem_poison
        self.nc.clear_and_free_semaphores(list(self.sems.allocated().values()))
        self.nc.all_engine_barrier()

    def __exit__(self, *a):
        r = super().__exit__(*a)
        _split_multi_waits(self.nc)
        return r


def build_program(use_b1, use_g2b2, use_cb, use_alpha,
                  jl512=(ND, ND), jltile=(ND,) * NT,
                  g2act=tuple((True,) * 4 for _ in range(NT))):
    nc = bass.Bass()
    dp = nc.declare_dram_parameter
    x_d = dp("x", [N, D], F32, isOutput=False)
    wT_d = dp("wT", [D, EXPD], BF16, isOutput=False)
    wcT_d = dp("wcT", [CIN, 2 * D], BF16, isOutput=False)
    im_d = dp("imask", [N, D], BF16, isOutput=False)
    om_d = dp("omask", [N, 2 * D], BF16, isOutput=False)
    mb_d = dp("mb", [4 * D], F32, isOutput=False)
    if use_alpha:
        psel_d = dp("psel", [N], F32, isOutput=False)
        alpha_d = dp("alpha", [1], F32, isOutput=False)
    if use_cb:
        cb_d = dp("cb", [2 * D], F32, isOutput=False)
    if use_g2b2:
        g2_d = dp("g2", [D], F32, isOutput=False)
        b2_d = dp("b2", [D], F32, isOutput=False)
    if use_b1:
        b4_d = dp("b4", [NE, EXPD], BF16, isOutput=False)
        ohT_d = dp("ohT", [NE, N], BF16, isOutput=False)
    out_d = dp("out", [N, D], F32, isOutput=True)
    # DRAM scratch for cross-layout bounces (row -> [128,8] -> broadcast)
    ksum_d = nc.dram_tensor("ksum_s", [N], F32)
    ksq_d = nc.dram_tensor("ksq_s", [N], F32)
    kmh_d = nc.dram_tensor("kmh_s", [N], F32)
    kih_d = nc.dram_tensor("kih_s", [N], F32)
    rs_d = nc.dram_tensor("rs_s", [H, N], F32)
    rr_d = nc.dram_tensor("rr_s", [H, N], F32)

    def COLI(v):
        return v.rearrange("(i p) -> p i", p=128)

    with TC(nc) as tc, ExitStack() as stk:
        # ------------------------------------------------ constants
        const = stk.enter_context(tc.tile_pool(name="const", bufs=1))
        mb_c = const.tile([128, NMLP], F32)
        nc.sync.dma_start(out=mb_c, in_=COLI(mb_d))
        eps_t = const.tile([128, 1], F32)
        nc.vector.memset(eps_t, EPS)
        zero_t = const.tile([128, 1], F32)
        nc.vector.memset(zero_t, 0.0)
        ones_bf = const.tile([128, 1], BF16)
        nc.vector.memset(ones_bf, 1.0)
        ident = const.tile([128, 128], BF16)
        make_identity(nc, ident)
        if use_alpha:
            psel_c = const.tile([128, NT], F32)
            nc.sync.dma_start(out=psel_c, in_=COLI(psel_d))
            alpha_c = const.tile([128, 1], F32)
            nc.sync.dma_start(out=alpha_c, in_=alpha_d[None, :].partition_broadcast(128))
            pscale_c = const.tile([128, NT], F32)
            nc.vector.tensor_scalar(pscale_c, psel_c, alpha_c, 1.0, OP.mult, OP.add)
        if use_cb:
            cb_c = const.tile([128, 2 * D], F32)
            nc.sync.dma_start(out=cb_c, in_=cb_d[None, :].partition_broadcast(128))
        if use_g2b2:
            g2_col = const.tile([128, ND], F32)
            nc.sync.dma_start(out=g2_col, in_=COLI(g2_d))
            b2_col = const.tile([128, ND], F32)
            nc.sync.dma_start(out=b2_col, in_=COLI(b2_d))
            g2_bc = const.tile([128, D], F32)
            nc.sync.dma_start(out=g2_bc, in_=g2_d[None, :].partition_broadcast(128))
            b2_bc = const.tile([128, D], F32)
            nc.sync.dma_start(out=b2_bc, in_=b2_d[None, :].partition_broadcast(128))
        if use_b1:
            b4_c = const.tile([NE, EXPD], BF16)
            nc.sync.dma_start(out=b4_c, in_=b4_d[:, :])
            ohT_c = const.tile([NE, N], BF16)
            nc.sync.dma_start(out=ohT_c, in_=ohT_d[:, :])

        # h^T, split so GEMM2 deps on attn vs mlp tiles stay independent
        hTp = stk.enter_context(tc.tile_pool(name="hTp", bufs=1))
        hTa = hTp.tile([128, ND, N], BF16)       # attn_out^T (c tiles 0..7)
        hTm1 = hTp.tile([128, 21, N], BF16)      # gelu(mlp)^T groups 0..20
        hTm2 = hTp.tile([128, NMLP - 21, N], BF16)  # gelu(mlp)^T groups 21..31

        def hTm_ap(m, isl=None):
            t = hTm1 if m < 21 else hTm2
            mm = m if m < 21 else m - 21
            if isl is None:
                return t[:, mm, :]
            return t[:, mm, isl]

        with tc.tile_pool(name="acts", bufs=1) as acts:
            xnT = acts.tile([128, ND, N], BF16)
            vtok = acts.tile([128, NT, D], BF16)
            kT = acts.tile([128, ND, N], BF16)

            # -------------------------------------------- LN1 + transpose
            with tc.tile_pool(name="ln1", bufs=3) as ln1, \
                 tc.tile_pool(name="tpp", bufs=4, space="PSUM") as tpp:
                for i in range(NT):
                    x_t = ln1.tile([128, D], F32, tag="xt")
                    nc.sync.dma_start(out=x_t, in_=x_d[i * 128:(i + 1) * 128, :])
                    im_t = ln1.tile([128, D], BF16, tag="im")
                    nc.sync.dma_start(out=im_t, in_=im_d[i * 128:(i + 1) * 128, :])
                    st = ln1.tile([128, 2, 6], F32, tag="st")
                    nc.vector.bn_stats(st[:, 0, :], x_t[:, 0:512])
                    nc.vector.bn_stats(st[:, 1, :], x_t[:, 512:1024])
                    mv = ln1.tile([128, 2], F32, tag="mv")
                    nc.vector.bn_aggr(mv, st)
                    istd = ln1.tile([128, 1], F32, tag="istd")
                    nc.scalar.activation(istd, mv[:, 1:2], AF.Sqrt, bias=eps_t)
                    nc.vector.reciprocal(istd, istd)
                    nbias = ln1.tile([128, 1], F32, tag="nbias")
                    nc.vector.tensor_scalar(nbias, mv[:, 0:1], istd, -1.0, OP.mult, OP.mult)
                    ln0 = ln1.tile([128, D], BF16, tag="ln0")
                    nc.scalar.activation(ln0, x_t, AF.Identity, bias=nbias, scale=istd)
                    xnm = ln1.tile([128, D], BF16, tag="xnm")
                    nc.vector.tensor_mul(xnm, ln0, im_t)
                    for g in range(2):
                        pt = tpp.tile([128, 512], BF16, tag="pt", name=f"pt{i}_{g}")
                        for jj in range(4):
                            j = 4 * g + jj
                            nc.tensor.transpose(
                                pt[:, jj * 128:(jj + 1) * 128],
                                xnm[:, j * 128:(j + 1) * 128], ident)
                        dst = xnT[:, 4 * g:4 * g + 4, i * 128:(i + 1) * 128]
                        src = pt.rearrange("p (j t) -> p j t", j=4)
                        if g == 0:
                            nc.vector.tensor_copy(dst, src)
                        else:
                            nc.scalar.copy(dst, src)

            with tc.tile_pool(name="g1w", bufs=3) as g1w, \
                 tc.tile_pool(name="wvp", bufs=1) as wvp:
                # prefetch the v weights early on the bulk queue
                wv = wvp.tile([128, ND, D], BF16)
                nc.sync.dma_start(
                    out=wv, in_=wT_d[:, 2 * D:3 * D].rearrange("(j p) o -> p j o", p=128))

                def load_w(o0):
                    w_t = g1w.tile([128, ND, 128], BF16, tag="w", name=f"w{o0}")
                    nc.sync.dma_start(
                        out=w_t,
                        in_=wT_d[:, o0:o0 + 128].rearrange("(j p) o -> p j o", p=128))
                    return w_t

                def fm_full(ps, w_t, o0):
                    for t2 in range(2):
                        jl = jl512[t2]
                        for j in range(jl):
                            nc.tensor.matmul(
                                ps[:, t2 * 512:(t2 + 1) * 512], w_t[:, j, :],
                                xnT[:, j, t2 * 512:(t2 + 1) * 512],
                                start=(j == 0), stop=(j == jl - 1 and not use_b1))
                        if use_b1:
                            nc.tensor.matmul(
                                ps[:, t2 * 512:(t2 + 1) * 512], b4_c[:, o0:o0 + 128],
                                ohT_c[:, t2 * 512:(t2 + 1) * 512],
                                start=False, stop=True)

                def fm_steps(ps, w_t, o0):
                    """K-step emitters for a feature-major group into psum [128,N]"""
                    def step(j):
                        for t2 in range(2):
                            jl = jl512[t2]
                            if j >= jl:
                                continue
                            nc.tensor.matmul(
                                ps[:, t2 * 512:(t2 + 1) * 512], w_t[:, j, :],
                                xnT[:, j, t2 * 512:(t2 + 1) * 512],
                                start=(j == 0), stop=(j == jl - 1 and not use_b1))
                    def tail():
                        if use_b1:
                            for t2 in range(2):
                                nc.tensor.matmul(
                                    ps[:, t2 * 512:(t2 + 1) * 512], b4_c[:, o0:o0 + 128],
                                    ohT_c[:, t2 * 512:(t2 + 1) * 512],
                                    start=False, stop=True)
                    return step, tail

                # ---- k tiles + LN2-k via PE column stats
                with tc.tile_pool(name="kps", bufs=2, space="PSUM") as kps, \
                     tc.tile_pool(name="ksps", bufs=1, space="PSUM") as ksps, \
                     tc.tile_pool(name="kbuf", bufs=1) as kbuf, \
                     tc.tile_pool(name="krow", bufs=1) as krow:
                    kraw = kbuf.tile([128, ND, N], BF16)
                    k2 = kbuf.tile([128, ND, N], BF16)
                    for rk in range(ND):
                        w_t = load_w(D + rk * 128)
                        ps = kps.tile([128, N], F32, tag="ps", name=f"kps{rk}")
                        fm_full(ps, w_t, D + rk * 128)
                        nc.scalar.copy(kraw[:, rk, :], ps)
                        nc.vector.tensor_mul(k2[:, rk, :], kraw[:, rk, :], kraw[:, rk, :])
                    ks_s = [ksps.tile([1, 512], F32, tag=f"s{t2}", name=f"ks_s{t2}")
                            for t2 in range(2)]
                    ks_q = [ksps.tile([1, 512], F32, tag=f"q{t2}", name=f"ks_q{t2}")
                            for t2 in range(2)]
                    for rk in range(ND):
                        for t2 in range(2):
                            nc.tensor.matmul(
                                ks_s[t2], ones_bf, kraw[:, rk, t2 * 512:(t2 + 1) * 512],
                                start=(rk == 0), stop=(rk == ND - 1))
                            nc.tensor.matmul(
                                ks_q[t2], ones_bf, k2[:, rk, t2 * 512:(t2 + 1) * 512],
                                start=(rk == 0), stop=(rk == ND - 1))
                    # finish stats in [128, 8] layout via DRAM bounce
                    srow = krow.tile([1, N], F32)
                    qrow = krow.tile([1, N], F32)
                    for t2 in range(2):
                        sl = slice(t2 * 512, (t2 + 1) * 512)
                        nc.vector.tensor_copy(srow[:, sl], ks_s[t2])
                        nc.vector.tensor_copy(qrow[:, sl], ks_q[t2])
                    nc.gpsimd.dma_start(out=ksum_d[None, :], in_=srow)
                    nc.gpsimd.dma_start(out=ksq_d[None, :], in_=qrow)
                    sc = krow.tile([128, NT], F32)
                    nc.sync.dma_start(out=sc, in_=COLI(ksum_d))
                    qc = krow.tile([128, NT], F32)
                    nc.sync.dma_start(out=qc, in_=COLI(ksq_d))
                    mc = krow.tile([128, NT], F32)
                    nc.vector.tensor_scalar(mc, sc, 1.0 / D, None, OP.mult)
                    m2c = krow.tile([128, NT], F32)
                    nc.vector.tensor_mul(m2c, mc, mc)
                    vc = krow.tile([128, NT], F32)
                    nc.vector.tensor_scalar(vc, qc, 1.0 / D, None, OP.mult)
                    nc.vector.tensor_sub(vc, vc, m2c)
                    ic = krow.tile([128, NT], F32)
                    nc.scalar.activation(ic, vc, AF.Sqrt, bias=eps_t)
                    nc.vector.reciprocal(ic, ic)
                    nc.gpsimd.dma_start(out=COLI(kmh_d), in_=mc)
                    nc.gpsimd.dma_start(out=COLI(kih_d), in_=ic)
                    km_f = krow.tile([128, N], F32)
                    nc.sync.dma_start(out=km_f, in_=kmh_d[None, :].partition_broadcast(128))
                    ki_f = krow.tile([128, N], F32)
                    nc.sync.dma_start(out=ki_f, in_=kih_d[None, :].partition_broadcast(128))
                    kmean_bc = krow.tile([128, N], BF16)
                    nc.vector.tensor_copy(kmean_bc, km_f)
                    kistd_bc = krow.tile([128, N], BF16)
                    nc.vector.tensor_copy(kistd_bc, ki_f)
                    with tc.tile_pool(name="ksb", bufs=2) as ksb:
                        for rk in range(ND):
                            t1 = ksb.tile([128, N], BF16, tag="kap")
                            nc.vector.tensor_sub(t1, kraw[:, rk, :], kmean_bc)
                            if use_g2b2:
                                t3 = ksb.tile([128, N], BF16, tag="kap2")
                                nc.vector.tensor_mul(t3, t1, kistd_bc)
                                nc.scalar.activation(kT[:, rk, :], t3, AF.Identity,
                                                     bias=b2_col[:, rk:rk + 1],
                                                     scale=g2_col[:, rk:rk + 1])
                            else:
                                nc.vector.tensor_mul(kT[:, rk, :], t1, kistd_bc)

                # ---- v tiles, token-major + LN2-v
                with tc.tile_pool(name="vps", bufs=2, space="PSUM") as vps, \
                     tc.tile_pool(name="vsb", bufs=2) as vsb:
                    for i in range(NT):
                        ps = vps.tile([128, D], F32, tag="ps", name=f"vps{i}")
                        jl = jltile[i]
                        for j in range(jl):
                            for c2 in range(2):
                                nc.tensor.matmul(
                                    ps[:, c2 * 512:(c2 + 1) * 512],
                                    xnT[:, j, i * 128:(i + 1) * 128],
                                    wv[:, j, c2 * 512:(c2 + 1) * 512],
                                    start=(j == 0), stop=(j == jl - 1 and not use_b1))
                        if use_b1:
                            for c2 in range(2):
                                nc.tensor.matmul(
                                    ps[:, c2 * 512:(c2 + 1) * 512],
                                    ohT_c[:, i * 128:(i + 1) * 128],
                                    b4_c[:, 2 * D + c2 * 512:2 * D + (c2 + 1) * 512],
                                    start=False, stop=True)
                        vf = vsb.tile([128, D], F32, tag="vf")
                        nc.vector.tensor_copy(vf, ps)
                        st = vsb.tile([128, 2, 6], F32, tag="vst")
                        nc.vector.bn_stats(st[:, 0, :], vf[:, 0:512])
                        nc.vector.bn_stats(st[:, 1, :], vf[:, 512:1024])
                        mv = vsb.tile([128, 2], F32, tag="vmv")
                        nc.vector.bn_aggr(mv, st)
                        vistd = vsb.tile([128, 1], F32, tag="vistd")
                        nc.scalar.activation(vistd, mv[:, 1:2], AF.Sqrt, bias=eps_t)
                        nc.vector.reciprocal(vistd, vistd)
                        if use_g2b2:
                            vln = vsb.tile([128, D], BF16, tag="vln")
                            nc.vector.tensor_scalar(vln, vf, mv[:, 0:1], vistd,
                                                    OP.subtract, OP.mult)
                            v2 = vsb.tile([128, D], BF16, tag="vln2")
                            nc.vector.tensor_mul(v2, vln, g2_bc)
                            nc.vector.tensor_add(vtok[:, i, :], v2, b2_bc)
                        else:
                            nc.vector.tensor_scalar(vtok[:, i, :], vf, mv[:, 0:1],
                                                    vistd, OP.subtract, OP.mult)

                # ---- attention head-pairs woven with mlp groups
                with tc.tile_pool(name="qkp", bufs=2, space="PSUM") as qkp, \
                     tc.tile_pool(name="mlpp", bufs=2, space="PSUM") as mlpp, \
                     tc.tile_pool(name="qp", bufs=2) as qpp, \
                     tc.tile_pool(name="est", bufs=2) as estp, \
                     tc.tile_pool(name="rsb", bufs=2) as rsbp:

                    def mlp_group_full(m):
                        w_t = load_w(3 * D + m * 128)
                        ps = mlpp.tile([128, N], F32, tag="mlp", name=f"mfull{m}")
                        fm_full(ps, w_t, 3 * D + m * 128)
                        nc.scalar.activation(hTm_ap(m), ps, AF.Identity,
                                             bias=mb_c[:, m:m + 1], scale=1.0)

                    def rs_head(hh, est_h):
                        rps = qkp.tile([128, N], F32, tag="qk", name=f"rs{hh}")
                        for t2 in range(2):
                            for j in range(ND):
                                nc.tensor.matmul(
                                    rps[0:1, t2 * 512:(t2 + 1) * 512], ones_bf,
                                    est_h[:, j, t2 * 512:(t2 + 1) * 512],
                                    start=(j == 0), stop=(j == ND - 1))
                        rrow = rsbp.tile([1, N], F32, tag="rrow", name=f"rr{hh}")
                        nc.vector.tensor_copy(rrow, rps[0:1, :])
                        nc.gpsimd.dma_start(out=rs_d[hh][None, :], in_=rrow)
                        rcol = rsbp.tile([128, NT], F32, tag="rcol", name=f"rc{hh}")
                        nc.sync.dma_start(out=rcol, in_=COLI(rs_d[hh]))
                        nc.vector.reciprocal(rcol, rcol)
                        nc.gpsimd.dma_start(out=COLI(rr_d[hh]), in_=rcol)

                    def attn_pair(hp):
                        h0, h1 = 2 * hp, 2 * hp + 1
                        # lazy q^T rows for this pair
                        wq_t = load_w(hp * 128)
                        psq = mlpp.tile([128, N], F32, tag="mlp", name=f"qg{hp}")
                        fm_full(psq, wq_t, hp * 128)
                        qp = qpp.tile([128, N], BF16, tag="q", name=f"qp{hp}")
                        nc.vector.tensor_copy(qp, psq)
                        # two mlp groups woven into the QK/exp pipeline
                        mA, mB = 3 * hp, 3 * hp + 1
                        wA = load_w(3 * D + mA * 128)
                        wB = load_w(3 * D + mB * 128)
                        psA = mlpp.tile([128, N], F32, tag="mlp", name=f"mA{hp}")
                        psB = mlpp.tile([128, N], F32, tag="mlp", name=f"mB{hp}")
                        stepA, tailA = fm_steps(psA, wA, 3 * D + mA * 128)
                        stepB, tailB = fm_steps(psB, wB, 3 * D + mB * 128)
                        ests = {}
                        for hh in (h0, h1):
                            ests[hh] = estp.tile([128, ND, N], BF16, tag="est",
                                                 name=f"est{hh}")
                        last_exp = None
                        for j in range(ND):
                            for hh, hb in ((h0, 0), (h1, 64)):
                                ps = qkp.tile([128, N], F32, tag="qk", name=f"qk{hh}_{j}")
                                for qc in range(2):
                                    nc.tensor.matmul(
                                        ps[:, qc * 512:(qc + 1) * 512],
                                        kT[hb:hb + 64, hp, j * 128:(j + 1) * 128],
                                        qp[hb:hb + 64, qc * 512:(qc + 1) * 512],
                                        start=True, stop=True)
                                last_exp = nc.scalar.activation(
                                    ests[hh][:, j, :], ps, AF.Exp, scale=1.0 / 8.0)
                            stepA(j)
                            stepB(j)
                        # previous pair's gelus go here, after this pair's exps,
                        # so the ACT table switch never lands inside the weave
                        if hp >= 1:
                            for m in range(3 * (hp - 1), 3 * hp):
                                g = nc.scalar.activation(hTm_ap(m), hTm_ap(m),
                                                         AF.Gelu, bias=zero_t, scale=1.0)
                                add_dep_helper(g.ins, last_exp.ins,
                                               reason="gelu after pair exps")
                        tailA()
                        tailB()
                        nc.scalar.activation(hTm_ap(mA), psA, AF.Identity,
                                             bias=mb_c[:, mA:mA + 1], scale=1.0)
                        nc.scalar.activation(hTm_ap(mB), psB, AF.Identity,
                                             bias=mb_c[:, mB:mB + 1], scale=1.0)
                        # third mlp group as PE filler while exps drain
                        mlp_group_full(3 * hp + 2)
                        rs_head(h0, ests[h0])
                        rs_head(h1, ests[h1])
                        # AV, col-tiled per head pair
                        rsb = rsbp.tile([128, N], F32, tag="rsb", name=f"rsb{hp}")
                        nc.sync.dma_start(out=rsb[0:64, :],
                                            in_=rr_d[h0][None, :].partition_broadcast(64))
                        nc.sync.dma_start(out=rsb[64:128, :],
                                            in_=rr_d[h1][None, :].partition_broadcast(64))
                        psv = qkp.tile([128, N], F32, tag="qk", name=f"av{hp}")
                        for t2 in range(2):
                            sl = slice(t2 * 512, (t2 + 1) * 512)
                            for j in range(NT):
                                nc.tensor.matmul(
                                    psv[0:64, sl], vtok[:, j, hp * 128:hp * 128 + 64],
                                    ests[h0][:, j, sl],
                                    start=(j == 0), stop=(j == NT - 1))
                                nc.tensor.matmul(
                                    psv[64:128, sl],
                                    vtok[:, j, hp * 128 + 64:hp * 128 + 128],
                                    ests[h1][:, j, sl],
                                    start=(j == 0), stop=(j == NT - 1),
                                    tile_position=(0, 64))
                        nc.vector.tensor_copy(hTa[:, hp, :], psv)

                    def gelu_batch(ms):
                        for m in ms:
                            nc.scalar.activation(hTm_ap(m), hTm_ap(m),
                                                 AF.Gelu, bias=zero_t, scale=1.0)

                    for hp in range(NT):
                        attn_pair(hp)
                    # tail mlp groups; their gelus overlap the start of GEMM2
                    for m in range(24, 28):
                        mlp_group_full(m)
                    gelu_batch(range(21, 24))
                    for m in range(28, 32):
                        mlp_group_full(m)
                    gelu_batch(range(24, 32))
                    # batched softmax normalization of the attention halves
                    for hp in range(NT):
                        rsb = rsbp.tile([128, N], F32, tag="rsb", name=f"rsbf{hp}")
                        nc.sync.dma_start(
                            out=rsb[0:64, :],
                            in_=rr_d[2 * hp][None, :].partition_broadcast(64))
                        nc.sync.dma_start(
                            out=rsb[64:128, :],
                            in_=rr_d[2 * hp + 1][None, :].partition_broadcast(64))
                        nc.vector.tensor_mul(hTa[:, hp, :], hTa[:, hp, :], rsb)

        # ------------------------------------------------ GEMM2 + combine
        # prefill out with x (residual base); masked z halves accumulate onto it
        nc.sync.dma_start(out=out_d[:, :], in_=x_d[:, :])
        # K-loop order: mlp tiles that gelu'd early first, attn tiles, last gelus
        jc_order = [8 + m for m in range(21)] + list(range(8)) + [8 + m for m in range(21, 32)]

        def lhs_g2(jc, i):
            isl = slice(i * 128, (i + 1) * 128)
            if jc < 8:
                return hTa[:, jc, isl]
            return hTm_ap(jc - 8, isl)

        with tc.tile_pool(name="g2w", bufs=2) as g2w, \
             tc.tile_pool(name="g2ps", bufs=6, space="PSUM") as g2ps, \
             tc.tile_pool(name="g2sb", bufs=6) as g2sb:
            for op2 in range(2):  # o2-chunk pairs: (z1a,z1b) then (z2a,z2b)
                wcs = []
                for oc in (2 * op2, 2 * op2 + 1):
                    w = g2w.tile([128, NC, 512], BF16, tag="wc", name=f"wc{oc}")
                    for jr in range(5):  # progressive 1MB sub-loads
                        nc.sync.dma_start(
                            out=w[:, jr * 8:(jr + 1) * 8, :],
                            in_=wcT_d[jr * 1024:(jr + 1) * 1024,
                                      oc * 512:(oc + 1) * 512].rearrange(
                                          "(j p) o -> p j o", p=128))
                    wcs.append(w)
                for i in range(NT):
                    act_us = [u for u in range(2) if g2act[i][2 * op2 + u]]
                    if not act_us:
                        continue
                    pss = {u: g2ps.tile([128, 512], F32, tag="z", name=f"z{op2}_{i}_{u}")
                           for u in act_us}
                    for nj, jc in enumerate(jc_order):
                        for u in act_us:
                            nc.tensor.matmul(pss[u], lhs_g2(jc, i), wcs[u][:, jc, :],
                                             start=(nj == 0), stop=(nj == NC - 1))
                    for u in act_us:
                        oc = 2 * op2 + u
                        om_t = g2sb.tile([128, 512], BF16, tag="om")
                        nc.sync.dma_start(
                            out=om_t,
                            in_=om_d[i * 128:(i + 1) * 128, oc * 512:(oc + 1) * 512])
                        zm = g2sb.tile([128, 512], F32, tag="zm")
                        if use_cb:
                            zb = g2sb.tile([128, 512], F32, tag="zb")
                            nc.vector.tensor_add(zb, pss[u], cb_c[:, oc * 512:(oc + 1) * 512])
                            nc.vector.tensor_mul(zm, zb, om_t)
                        else:
                            nc.vector.tensor_mul(zm, pss[u], om_t)
                        if op2 == 1 and use_alpha:
                            zs = g2sb.tile([128, 512], F32, tag="zs")
                            nc.vector.tensor_scalar(zs, zm, pscale_c[:, i:i + 1], None,
                                                    OP.mult)
                            zm = zs
                        dcol = (oc - 2) * 512 if op2 == 1 else oc * 512
                        nc.gpsimd.dma_start(
                            out=out_d[i * 128:(i + 1) * 128, dcol:dcol + 512],
                            in_=zm, accum_op=OP.add)
    return nc


_PROG_CACHE = {}


def prepare(x, expert_mask, router_probs, expand_weight, mlp_bias,
            contract_weight, contract_bias, norm1_g, norm1_b,
            norm2_g, norm2_b, alpha):
    """Build (program, per-core input maps) for the given full inputs."""
    x = np.asarray(x, np.float32)
    expert_mask = np.asarray(expert_mask, np.int32)
    router_probs = np.asarray(router_probs, np.float32)
    W = np.asarray(expand_weight, np.float32)
    mb = np.asarray(mlp_bias, np.float32)
    Wc = np.asarray(contract_weight, np.float32)
    cb = np.asarray(contract_bias, np.float32)
    g1 = np.asarray(norm1_g, np.float32)
    b1 = np.asarray(norm1_b, np.float32)
    g2 = np.asarray(norm2_g, np.float32)
    b2 = np.asarray(norm2_b, np.float32)
    alpha = np.asarray(alpha, np.float32)

    use_b1 = bool(np.any(b1 != 0))
    use_g2b2 = bool(np.any(b2 != 0) or np.any(g2 != 1))
    use_cb = bool(np.any(cb != 0))
    use_alpha = bool(np.any(alpha != 0))

    # sort tokens by expert (attention is permutation-equivariant) so the
    # nested-dims masks become block patterns the program can skip exactly
    perm = np.argsort(expert_mask, axis=-1, kind="stable")          # [B, N]
    inv = np.argsort(perm, axis=-1, kind="stable")
    em_s = np.take_along_axis(expert_mask, perm, axis=-1)
    shift = (NE - 1 - em_s).astype(np.int64)
    d_in = (D >> shift)                    # [B, N] sorted
    d_out = ((2 * D) >> shift)             # [B, N] sorted
    # conservative (max over cores) per-chunk contraction limits
    din_max_t = d_in.reshape(B, NT, 128).max(axis=(0, 2))           # per 128-tile
    jltile = tuple(int(-(-v // 128)) for v in din_max_t)
    jl512 = tuple(max(jltile[4 * c:4 * c + 4]) for c in range(2))
    dout_max_t = d_out.reshape(B, NT, 128).max(axis=(0, 2))
    g2act = tuple(tuple(bool(dout_max_t[i] > oc * 512) for oc in range(4))
                  for i in range(NT))

    key = (use_b1, use_g2b2, use_cb, use_alpha, jl512, jltile, g2act)
    if key not in _PROG_CACHE:
        _PROG_CACHE[key] = build_program(use_b1, use_g2b2, use_cb, use_alpha,
                                         jl512, jltile, g2act)
    nc = _PROG_CACHE[key]

    wT = np.ascontiguousarray((W * g1[None, :]).T).astype(ml_dtypes.bfloat16)
    wcT = np.ascontiguousarray(Wc.T).astype(ml_dtypes.bfloat16)
    imask = (np.arange(D)[None, None, :] < d_in[..., None]).astype(ml_dtypes.bfloat16)
    omask = (np.arange(2 * D)[None, None, :] < d_out[..., None]).astype(ml_dtypes.bfloat16)
    psel_full = np.take_along_axis(router_probs, expert_mask[..., None], axis=-1)[..., 0]
    psel = np.take_along_axis(psel_full, perm, axis=-1)
    x_s = np.take_along_axis(x, perm[..., None], axis=1)

    in_maps = []
    for b in range(B):
        m = dict(x=np.ascontiguousarray(x_s[b]), wT=wT, wcT=wcT, mb=mb,
                 imask=np.ascontiguousarray(imask[b]),
                 omask=np.ascontiguousarray(omask[b]))
        if use_alpha:
            m["psel"] = np.ascontiguousarray(psel[b].astype(np.float32))
            m["alpha"] = alpha
        if use_cb:
            m["cb"] = cb
        if use_g2b2:
            m["g2"] = g2
            m["b2"] = b2
        if use_b1:
            mask_e = (np.arange(D)[None, :] < (D >> (NE - 1 - np.arange(NE)))[:, None])
            b4 = ((b1[None, :] * mask_e) @ W.T).astype(ml_dtypes.bfloat16)
            m["b4"] = b4
            ohT = np.zeros((NE, N), np.float32)
            ohT[em_s[b], np.arange(N)] = 1.0
            m["ohT"] = ohT.astype(ml_dtypes.bfloat16)
        in_maps.append(m)
    return nc, in_maps, inv


def kernel(x, expert_mask, router_probs, expand_weight, mlp_bias,
           contract_weight, contract_bias, norm1_g, norm1_b,
           norm2_g, norm2_b, alpha):
    nc, in_maps, inv = prepare(x, expert_mask, router_probs, expand_weight,
                               mlp_bias, contract_weight, contract_bias,
                               norm1_g, norm1_b, norm2_g, norm2_b, alpha)
    res = run_bass_kernel_spmd(nc, in_maps, list(range(B)))
    out_s = np.stack([res.results[b]["out"] for b in range(B)], axis=0)
    out = np.take_along_axis(out_s, np.asarray(inv)[..., None], axis=1)
    return (out, np.asarray(expert_mask, np.int32),
            np.asarray(router_probs, np.float32))
